# revision 12
# baseline (speedup 1.0000x reference)
"""Baichuan attention layer (B=1, S=2048, E=4096, H=32, D=128) on 8 Trainium2
NeuronCores.

Sharding:
- QKV projection + RoPE + causal attention: tensor-parallel by head (4 heads
  per core). All per-head tensors live in transposed [feature, seq] layout so
  every matmul contracts over the partition dim with zero transposes:
    qkv^T[f, s]   = W @ X^T                (lhsT = W^T tiles, rhs = X^T tiles)
    scores^T[k,q] = K @ Q^T                (lhsT = K^T tile, rhs = Q^T block)
    att^T[d, q]   = V^T @ P^T              (lhsT = V tile,   rhs = exp tile)
  Softmax runs without max-subtraction (scores ~ N(0,1) after 1/sqrt(D), fp32
  exp is safe).  The denominator is accumulated on the Vector engine (exp
  tiles summed elementwise in fp32), then one all-ones [128,128] lhsT matmul
  per (head, q-block) replicates the k-sum across all PSUM partitions.
  RoPE's rotate-half is a partition swap done by an SBUF->SBUF DMA, with the
  sign folded into a host-precomputed signed-sin table; no PE matmul needed.
- One AllGather of att^T [512, 2048] bf16 per core -> full att^T [4096, 2048].
- o_proj: column-parallel (each core computes its 512 output columns for the
  full sequence, using its slice of w_o). Host concatenates along E.

All matmuls in bf16 with fp32 PSUM accumulation.
"""

import importlib.util
import sys
import types

import numpy as np
import ml_dtypes

BF16NP = ml_dtypes.bfloat16

B, S, E = 1, 2048, 4096
H, D = 32, 128
NCORES = 8
HPC = H // NCORES          # heads per core = 4
P = 128                    # partitions
SBLK = 512                 # seq block (matmul free dim)
NSBLK = S // SBLK          # 4
ET = E // P                # 32 e-tiles
NF = 3 * HPC               # 12 f-tiles per core (q0..3, k4..7, v8..11)
KT = S // P                # 16 k-tiles
ECOLS = E // NCORES        # 512 output columns per core
SCALE = 1.0 / float(np.sqrt(D))
HALF = D // 2


def _install_ntff_hook():
    """antenv.axon_hooks is absent in this image; recreate it from trn_boot's
    ctypes shim so run_bass_kernel_spmd(trace=True) can capture NTFF traces."""
    if "antenv.axon_hooks" in sys.modules:
        return
    try:
        spec = importlib.util.spec_from_file_location(
            "trn_boot", "/root/.axon_site/trn_agent_boot/trn_boot.py")
        tb = importlib.util.module_from_spec(spec)
        spec.loader.exec_module(tb)
        hook = tb._ntff_profile_via_ctypes("/opt/axon/libaxon_pjrt.so")
    except Exception:
        hook = None
    mod = types.ModuleType("antenv.axon_hooks")
    mod.get_axon_ntff_profile_hook = lambda: hook
    mod.set_axon_ntff_profile_hook = lambda h: None
    sys.modules["antenv.axon_hooks"] = mod


_install_ntff_hook()

import concourse.bass as bass  # noqa: E402
import concourse.mybir as mybir  # noqa: E402
import concourse.tile as tile  # noqa: E402
from concourse import bacc  # noqa: E402
from concourse.bass import ts  # noqa: E402
from concourse.bass_utils import run_bass_kernel_spmd  # noqa: E402

BF16 = mybir.dt.bfloat16
F32 = mybir.dt.float32

_NC_CACHE = None


def build():
    global _NC_CACHE
    if _NC_CACHE is not None:
        return _NC_CACHE
    nc = bacc.Bacc("TRN2", target_bir_lowering=False, debug=False,
                   num_devices=NCORES)

    xt_ext = nc.dram_tensor("xt", [E, S], BF16, kind="ExternalInput")
    wt_ext = nc.dram_tensor("wt", [E, NF * P], BF16, kind="ExternalInput")
    wot_ext = nc.dram_tensor("wot", [E, ECOLS], BF16, kind="ExternalInput")
    cost_ext = nc.dram_tensor("cost", [D, S], F32, kind="ExternalInput")
    sins_ext = nc.dram_tensor("sins", [D, S], F32, kind="ExternalInput")
    masks_ext = nc.dram_tensor("masks", [4, P, SBLK], BF16, kind="ExternalInput")
    ones_ext = nc.dram_tensor("ones", [P, P], BF16, kind="ExternalInput")
    ident_ext = nc.dram_tensor("ident", [P, P], BF16, kind="ExternalInput")
    out_ext = nc.dram_tensor("out", [S, ECOLS], F32, kind="ExternalOutput")

    # One AllGather per (local head, q-block) chunk, issued as soon as that
    # block's attention output is ready: they overlap attention/o_proj
    # compute, and the fine granularity lets o_proj start on a head's early
    # q-blocks while later blocks are still in flight.  ccout[h][j] rank-r
    # block = rows [128r, 128r+128) = global head 4r + h, seq block j.
    ccins = [[nc.dram_tensor(f"ccin{h}_{j}", [P, SBLK], BF16)
              for j in range(NSBLK)] for h in range(HPC)]
    ccouts = [[nc.dram_tensor(f"ccout{h}_{j}", [NCORES * P, SBLK], BF16,
                              addr_space="Shared") for j in range(NSBLK)]
              for h in range(HPC)]

    xt_t = xt_ext.ap().rearrange("(eo p) s -> p eo s", p=P)
    wt_t = wt_ext.ap().rearrange("(eo p) f -> p eo f", p=P)
    wot_t = wot_ext.ap().rearrange("(fo p) e -> p fo e", p=P)
    masks_t = masks_ext.ap().rearrange("r p q -> p r q")
    # [p, c, s]: block c of ccout[h][j] = global head 4c + h
    ccout_ts = [[cc.ap().rearrange("(c p) s -> p c s", p=P) for cc in row]
                for row in ccouts]

    with tile.TileContext(nc) as tc:
        with (
            tc.tile_pool(name="cst", bufs=1) as cst,
            tc.tile_pool(name="ropeT", bufs=1) as ropeT_pool,
            tc.tile_pool(name="vall", bufs=1) as vall_pool,
        ):
            # q^T and k^T after RoPE: [128, 8, 2048]
            ropeT_sb = ropeT_pool.tile([P, 2 * HPC, S], BF16)
            # V tiles, transposed to [s, d] per 128x128 tile: [128, 64, 128]
            v_all_sb = vall_pool.tile([P, HPC * KT, P], BF16)

            # ---------------- Phase 1: QKV projection + RoPE -------------
            with (
                tc.tile_pool(name="xt", bufs=2) as xt_pool,
                tc.tile_pool(name="wq", bufs=5) as w_pool,
                tc.tile_pool(name="cs", bufs=2) as cs_pool,
                tc.tile_pool(name="qkc", bufs=3) as qkc_pool,
                tc.tile_pool(name="rot", bufs=3) as rot_pool,
                tc.tile_pool(name="rtmp", bufs=2) as rtmp_pool,
                tc.tile_pool(name="ps_qkv", bufs=3, space="PSUM") as ps_qkv,
                tc.tile_pool(name="ps_vtr", bufs=3, space="PSUM") as ps_vtr,
            ):
                # Pre-issue the b=0 input DMAs (chunked) so the first matmuls
                # start as early as possible; constants go afterwards.
                xt_tiles = {}
                w_tiles = {}
                xt_sb0 = xt_pool.tile([P, ET, SBLK], BF16, tag="xt")
                w_sb0 = w_pool.tile([P, ET, P], BF16, tag="w")
                for ch in range(8):
                    nc.sync.dma_start(w_sb0[:, ts(ch, ET // 8), :],
                                      wt_t[:, ts(ch, ET // 8), ts(0, P)])
                    nc.sync.dma_start(xt_sb0[:, ts(ch, ET // 8), :],
                                      xt_t[:, ts(ch, ET // 8), ts(0, SBLK)])
                xt_tiles[0] = xt_sb0
                w_tiles[(0, 0)] = w_sb0

                # constants (needed later than the first matmuls)
                ones_sb = cst.tile([P, P], BF16)
                nc.sync.dma_start(ones_sb[:], ones_ext.ap())
                ident_sb = cst.tile([P, P], BF16)
                nc.sync.dma_start(ident_sb[:], ident_ext.ap())
                masks_sb = cst.tile([P, 4, SBLK], BF16)
                nc.sync.dma_start(masks_sb[:], masks_t)

                for b in range(NSBLK):
                    sblk = ts(b, SBLK)
                    xt_sb = xt_tiles.pop(b)
                    cos_sb = cs_pool.tile([D, SBLK], F32, tag="cos")
                    nc.sync.dma_start(cos_sb[:], cost_ext.ap()[:, sblk])
                    sin_sb = cs_pool.tile([D, SBLK], F32, tag="sin")
                    nc.sync.dma_start(sin_sb[:], sins_ext.ap()[:, sblk])

                    for f in range(NF):
                        if f == 3 and b + 1 < NSBLK:
                            # prefetch the next seq block early so its first
                            # matmuls don't wait at the block boundary
                            nxt = xt_pool.tile([P, ET, SBLK], BF16, tag="xt")
                            for ch in range(4):
                                nc.sync.dma_start(
                                    nxt[:, ts(ch, ET // 4), :],
                                    xt_t[:, ts(ch, ET // 4), ts(b + 1, SBLK)])
                            xt_tiles[b + 1] = nxt
                        if (b, f) in w_tiles:
                            w_sb = w_tiles[(b, f)]
                        else:
                            w_sb = w_pool.tile([P, ET, P], BF16, tag="w")
                            nc.sync.dma_start(w_sb[:], wt_t[:, :, ts(f, P)])
                        acc_ps = ps_qkv.tile([P, SBLK], F32, tag="qkv")
                        for e in range(ET):
                            nc.tensor.matmul(
                                acc_ps[:], w_sb[:, e, :], xt_sb[:, e, :],
                                start=(e == 0), stop=(e == ET - 1),
                            )
                        if f < 2 * HPC:
                            # q/k: RoPE.  rotate-half = partition swap (DMA)
                            # with the sign folded into the signed-sin table.
                            qk_sb = qkc_pool.tile([P, SBLK], BF16, tag="qkc")
                            nc.any.tensor_copy(qk_sb[:], acc_ps[:])
                            rot_sb = rot_pool.tile([P, SBLK], BF16, tag="rot")
                            nc.sync.dma_start(rot_sb[0:HALF, :],
                                              qk_sb[HALF:P, :])
                            nc.sync.dma_start(rot_sb[HALF:P, :],
                                              qk_sb[0:HALF, :])
                            t1 = rtmp_pool.tile([P, SBLK], F32, tag="t1")
                            nc.vector.tensor_mul(out=t1[:], in0=acc_ps[:],
                                                 in1=cos_sb[:])
                            t2 = rtmp_pool.tile([P, SBLK], F32, tag="t2")
                            nc.vector.tensor_mul(out=t2[:], in0=rot_sb[:],
                                                 in1=sin_sb[:])
                            nc.vector.tensor_add(
                                out=ropeT_sb[:, f, sblk], in0=t1[:], in1=t2[:])
                        else:
                            # v: bf16 copy, then transpose the four 128x128
                            # tiles inline (keeps the PE warm vs. a separate
                            # transpose block between phases)
                            h = f - 2 * HPC
                            v_sb = qkc_pool.tile([P, SBLK], BF16, tag="vst")
                            nc.any.tensor_copy(v_sb[:], acc_ps[:])
                            for t in range(4):
                                vt_ps = ps_vtr.tile([P, P], BF16, tag="vtr")
                                nc.tensor.transpose(vt_ps[:],
                                                    v_sb[:, ts(t, P)],
                                                    ident_sb[:])
                                nc.any.tensor_copy(
                                    v_all_sb[:, h * KT + 4 * b + t, :],
                                    vt_ps[:])

            # --- Phase 2+3: attention per head, with the previous head's
            # o_proj pass interleaved at matmul granularity.  The o_proj
            # matmuls are pure-PE work that hides the attention phase's
            # Scalar (exp) and Vector (denominator) load; without the
            # interleave, attention is Scalar/DVE-bound and PE idles ~40%.
            with (
                tc.tile_pool(name="wot", bufs=1) as wot_pool,
                tc.tile_pool(name="attnT", bufs=1) as attnT_pool,
                tc.tile_pool(name="exp", bufs=8) as exp_pool,
                tc.tile_pool(name="dsum", bufs=3) as dsum_pool,
                tc.tile_pool(name="dbf", bufs=2) as dbf_pool,
                tc.tile_pool(name="rcp", bufs=2) as rcp_pool,
                tc.tile_pool(name="at", bufs=4) as at_pool,
                tc.tile_pool(name="stage", bufs=4) as stage_pool,
                tc.tile_pool(name="osb", bufs=3) as osb_pool,
                tc.tile_pool(name="part", bufs=1) as part_pool,
                tc.tile_pool(name="ps_sc", bufs=3, space="PSUM") as ps_sc,
                tc.tile_pool(name="ps_av", bufs=2, space="PSUM") as ps_av,
                tc.tile_pool(name="ps_den", bufs=1, space="PSUM") as ps_den,
                tc.tile_pool(name="ps_out", bufs=2, space="PSUM") as ps_out,
            ):
                wot_sb = wot_pool.tile([P, ET, ECOLS], BF16)
                nc.sync.dma_start(wot_sb[:], wot_t)
                attnT_sb = attnT_pool.tile([P, HPC, S], BF16)
                NST = S // P
                part_sb = part_pool.tile([P, NST, ECOLS], F32)

                def op_tile(p_h, st):
                    # one o_proj seq-tile of pass p_h: [128 seq, 512 e-cols],
                    # contracting the 8 128-row blocks of ccout[p_h][st//4].
                    # PSUM is evacuated by the Scalar engine into an SBUF
                    # stage so the PE never waits on the (busy) Vector
                    # engine; the part-sum add runs on DVE off the critical
                    # path.
                    a_sb = at_pool.tile([P, 8, P], BF16, tag="at")
                    nc.sync.dma_start(
                        a_sb[:], ccout_ts[p_h][st // 4][:, :, ts(st % 4, P)])
                    o_ps = ps_out.tile([P, ECOLS], F32, tag="out")
                    for c in range(8):
                        nc.tensor.matmul(o_ps[:], a_sb[:, c, :],
                                         wot_sb[:, 4 * c + p_h, :],
                                         start=(c == 0), stop=(c == 7))
                    if p_h == 0:
                        nc.scalar.copy(part_sb[:, st, :], o_ps[:])
                    else:
                        stg = stage_pool.tile([P, ECOLS], F32, tag="stg")
                        nc.scalar.copy(stg[:], o_ps[:])
                        if p_h < HPC - 1:
                            nc.vector.tensor_add(out=part_sb[:, st, :],
                                                 in0=part_sb[:, st, :],
                                                 in1=stg[:])
                        else:
                            o_sb = osb_pool.tile([P, ECOLS], F32, tag="osb")
                            nc.vector.tensor_add(out=o_sb[:], in0=stg[:],
                                                 in1=part_sb[:, st, :])
                            nc.sync.dma_start(out_ext.ap()[ts(st, P), :],
                                              o_sb[:])

                # attention head h has 40 score tiles; the 16 o_proj tiles
                # of pass h-1 slot in every other tile from tile 10 on (the
                # per-block AllGathers of head h-1 have all landed by then).
                OP_START = 10

                for h in range(HPC):
                    v_sb = v_all_sb[:, h * KT:(h + 1) * KT, :]
                    qh = ropeT_sb[:, h, :]
                    kh = ropeT_sb[:, HPC + h, :]
                    op_queue = [(h - 1, st) for st in range(NST)] if h else []
                    icount = 0
                    for j in range(NSBLK):
                        nkt = 4 * j + 4
                        av_ps = ps_av.tile([P, SBLK], F32, tag="av")
                        dsum = dsum_pool.tile([P, SBLK], F32, tag="dsum")
                        for i in range(nkt):
                            # diagonal tile r: columns below 128r are
                            # fully masked -> compute only [off:SBLK]
                            r = i - 4 * j
                            off = 128 * r if r > 0 else 0
                            qs = bass.ds(j * SBLK + off, SBLK - off)
                            sc_ps = ps_sc.tile([P, SBLK], F32, tag="sc")
                            nc.tensor.matmul(sc_ps[:, off:], kh[:, ts(i, P)],
                                             qh[:, qs],
                                             start=True, stop=True)
                            exp_sb = exp_pool.tile([P, SBLK], BF16,
                                                   tag="exp")
                            nc.scalar.activation(
                                exp_sb[:, off:], sc_ps[:, off:],
                                mybir.ActivationFunctionType.Exp,
                                scale=SCALE)
                            if r >= 0:
                                nc.vector.tensor_mul(
                                    out=exp_sb[:, off:],
                                    in0=exp_sb[:, off:],
                                    in1=masks_sb[:, r, off:])
                            # denominator partial sums accumulate on DVE
                            # (fp32); one matmul per (h, j) replicates the
                            # k-sum across partitions afterwards.
                            if i == 0:
                                nc.vector.tensor_copy(dsum[:], exp_sb[:])
                            else:
                                nc.vector.tensor_add(
                                    out=dsum[:, off:], in0=dsum[:, off:],
                                    in1=exp_sb[:, off:])
                            nc.tensor.matmul(
                                av_ps[:, off:], v_sb[:, i, :],
                                exp_sb[:, off:],
                                start=(i == 0), stop=(i == nkt - 1))
                            icount += 1
                            if (op_queue and icount >= OP_START
                                    and icount % 2 == 0):
                                op_tile(*op_queue.pop(0))
                        dbf = dbf_pool.tile([P, SBLK], BF16, tag="dbf")
                        nc.vector.tensor_copy(dbf[:], dsum[:])
                        den_ps = ps_den.tile([P, SBLK], F32, tag="den")
                        nc.tensor.matmul(den_ps[:], ones_sb[:], dbf[:],
                                         start=True, stop=True)
                        recip_sb = rcp_pool.tile([P, SBLK], F32,
                                                 tag="rcp")
                        nc.vector.reciprocal_approx_fast(
                            out=recip_sb[:], in_=den_ps[:])
                        nc.vector.tensor_mul(
                            out=attnT_sb[:, h, ts(j, SBLK)],
                            in0=av_ps[:], in1=recip_sb[:])
                        # ship this q-block immediately: the chunked
                        # AllGather overlaps the rest of the head and lets
                        # o_proj consume early blocks of the final head
                        # without waiting for the whole head to finish.
                        nc.sync.dma_start(
                            ccins[h][j].ap().rearrange("(o p) s -> p o s",
                                                       p=P),
                            attnT_sb[:, h:h + 1, ts(j, SBLK)])
                        nc.gpsimd.collective_compute(
                            "AllGather", mybir.AluOpType.bypass,
                            replica_groups=[list(range(NCORES))],
                            ins=[ccins[h][j].ap()],
                            outs=[ccouts[h][j].ap()],
                        )

                    while op_queue:
                        op_tile(*op_queue.pop(0))

                # final o_proj pass (local head 3) after its AllGathers
                for st in range(NST):
                    op_tile(HPC - 1, st)

    nc.compile()
    _NC_CACHE = nc
    return nc


def _prep_inputs(hidden_states, cos, sin, w_pack, w_o):
    hs = np.asarray(hidden_states, dtype=np.float32).reshape(S, E)
    xt = np.ascontiguousarray(hs.T).astype(BF16NP)
    cost = np.ascontiguousarray(np.asarray(cos, dtype=np.float32).T)
    sint = np.ascontiguousarray(np.asarray(sin, dtype=np.float32).T)
    # signed sin table: rotate_half's sign folded in (rows 0..63 negated)
    sins = sint.copy()
    sins[:HALF] = -sins[:HALF]
    w_pack = np.asarray(w_pack, dtype=np.float32)
    w_o = np.asarray(w_o, dtype=np.float32)

    masks = np.zeros((4, P, SBLK), dtype=np.float32)
    kk = np.arange(P)[:, None]
    qq = np.arange(SBLK)[None, :]
    for r in range(4):
        masks[r] = (P * r + kk <= qq).astype(np.float32)
    masks = masks.astype(BF16NP)

    ones = np.ones((P, P), dtype=BF16NP)
    ident = np.eye(P, dtype=np.float32).astype(BF16NP)

    in_maps = []
    hw = E // NCORES  # 512 head-rows per core in each of q/k/v
    for c in range(NCORES):
        rows = slice(c * hw, (c + 1) * hw)
        wqkv = np.concatenate(
            [w_pack[rows], w_pack[E:][rows], w_pack[2 * E:][rows]], axis=0)
        wt = np.ascontiguousarray(wqkv.T).astype(BF16NP)
        wot = np.ascontiguousarray(w_o[rows].T).astype(BF16NP)
        in_maps.append({
            "xt": xt, "wt": wt, "wot": wot,
            "cost": cost, "sins": sins,
            "masks": masks, "ones": ones, "ident": ident,
        })
    return in_maps


def run(trace=False, trace_cores=None, **inputs):
    nc = build()
    in_maps = _prep_inputs(**inputs)
    res = run_bass_kernel_spmd(
        nc, in_maps, core_ids=list(range(NCORES)),
        trace=trace, trace_cores=trace_cores,
    )
    out = np.concatenate([res.results[c]["out"] for c in range(NCORES)], axis=1)
    return out.reshape(B, S, E).astype(np.float32), res


def kernel(**inputs) -> np.ndarray:
    out, _ = run(trace=False, **inputs)
    return out


# revision 20
# speedup vs baseline: 1.0399x; 1.0399x over previous
"""Baichuan attention layer (B=1, S=2048, E=4096, H=32, D=128) on 8 Trainium2
NeuronCores.

Sharding:
- QKV projection + RoPE + causal attention: tensor-parallel by head (4 heads
  per core). All per-head tensors live in transposed [feature, seq] layout so
  every matmul contracts over the partition dim with zero transposes:
    qkv^T[f, s]   = W @ X^T                (lhsT = W^T tiles, rhs = X^T tiles)
    scores^T[k,q] = K @ Q^T                (lhsT = K^T tile, rhs = Q^T block)
    att^T[d, q]   = V^T @ P^T              (lhsT = V tile,   rhs = exp tile)
  Softmax runs without max-subtraction (scores ~ N(0,1) after 1/sqrt(D), fp32
  exp is safe).  The denominator is accumulated on the Vector engine (exp
  tiles summed elementwise in fp32), then one all-ones [128,128] lhsT matmul
  per (head, q-block) replicates the k-sum across all PSUM partitions.
  RoPE's rotate-half is a partition swap done by an SBUF->SBUF DMA, with the
  sign folded into a host-precomputed signed-sin table; no PE matmul needed.
- One AllGather of att^T [512, 2048] bf16 per core -> full att^T [4096, 2048].
- o_proj: column-parallel (each core computes its 512 output columns for the
  full sequence, using its slice of w_o). Host concatenates along E.

All matmuls in bf16 with fp32 PSUM accumulation.
"""

import importlib.util
import sys
import types

import numpy as np
import ml_dtypes

BF16NP = ml_dtypes.bfloat16

B, S, E = 1, 2048, 4096
H, D = 32, 128
NCORES = 8
HPC = H // NCORES          # heads per core = 4
P = 128                    # partitions
SBLK = 512                 # seq block (matmul free dim)
NSBLK = S // SBLK          # 4
ET = E // P                # 32 e-tiles
NF = 3 * HPC               # 12 f-tiles per core (q0..3, k4..7, v8..11)
KT = S // P                # 16 k-tiles
ECOLS = E // NCORES        # 512 output columns per core
SCALE = 1.0 / float(np.sqrt(D))
HALF = D // 2


def _install_ntff_hook():
    """antenv.axon_hooks is absent in this image; recreate it from trn_boot's
    ctypes shim so run_bass_kernel_spmd(trace=True) can capture NTFF traces."""
    if "antenv.axon_hooks" in sys.modules:
        return
    try:
        spec = importlib.util.spec_from_file_location(
            "trn_boot", "/root/.axon_site/trn_agent_boot/trn_boot.py")
        tb = importlib.util.module_from_spec(spec)
        spec.loader.exec_module(tb)
        hook = tb._ntff_profile_via_ctypes("/opt/axon/libaxon_pjrt.so")
    except Exception:
        hook = None
    mod = types.ModuleType("antenv.axon_hooks")
    mod.get_axon_ntff_profile_hook = lambda: hook
    mod.set_axon_ntff_profile_hook = lambda h: None
    sys.modules["antenv.axon_hooks"] = mod


_install_ntff_hook()

import concourse.bass as bass  # noqa: E402
import concourse.mybir as mybir  # noqa: E402
import concourse.tile as tile  # noqa: E402
from concourse import bacc  # noqa: E402
from concourse.bass import ts  # noqa: E402
from concourse.bass_utils import run_bass_kernel_spmd  # noqa: E402

BF16 = mybir.dt.bfloat16
F32 = mybir.dt.float32

_NC_CACHE = None


def build():
    global _NC_CACHE
    if _NC_CACHE is not None:
        return _NC_CACHE
    nc = bacc.Bacc("TRN2", target_bir_lowering=False, debug=False,
                   num_devices=NCORES)

    xt_ext = nc.dram_tensor("xt", [E, S], BF16, kind="ExternalInput")
    wt_ext = nc.dram_tensor("wt", [E, NF * P], BF16, kind="ExternalInput")
    wot_ext = nc.dram_tensor("wot", [E, ECOLS], BF16, kind="ExternalInput")
    cost_ext = nc.dram_tensor("cost", [D, S], F32, kind="ExternalInput")
    sins_ext = nc.dram_tensor("sins", [D, S], F32, kind="ExternalInput")
    rt_ext = nc.dram_tensor("rt", [D, D], BF16, kind="ExternalInput")
    masks_ext = nc.dram_tensor("masks", [4, P, SBLK], BF16, kind="ExternalInput")
    ones_ext = nc.dram_tensor("ones", [P, P], BF16, kind="ExternalInput")
    ident_ext = nc.dram_tensor("ident", [P, P], BF16, kind="ExternalInput")
    out_ext = nc.dram_tensor("out", [S, ECOLS], F32, kind="ExternalOutput")

    # One AllGather per (local head, q-block) chunk, issued as soon as that
    # block's attention output is ready: they overlap attention/o_proj
    # compute, and the fine granularity lets o_proj start on a head's early
    # q-blocks while later blocks are still in flight.  ccout[h][j] rank-r
    # block = rows [128r, 128r+128) = global head 4r + h, seq block j.
    ccins = [[nc.dram_tensor(f"ccin{h}_{j}", [P, SBLK], BF16)
              for j in range(NSBLK)] for h in range(HPC)]
    ccouts = [[nc.dram_tensor(f"ccout{h}_{j}", [NCORES * P, SBLK], BF16,
                              addr_space="Shared") for j in range(NSBLK)]
              for h in range(HPC)]

    xt_t = xt_ext.ap().rearrange("(eo p) s -> p eo s", p=P)
    wt_t = wt_ext.ap().rearrange("(eo p) f -> p eo f", p=P)
    wot_t = wot_ext.ap().rearrange("(fo p) e -> p fo e", p=P)
    masks_t = masks_ext.ap().rearrange("r p q -> p r q")
    # [p, c, s]: block c of ccout[h][j] = global head 4c + h
    ccout_ts = [[cc.ap().rearrange("(c p) s -> p c s", p=P) for cc in row]
                for row in ccouts]

    with tile.TileContext(nc) as tc:
        with (
            tc.tile_pool(name="cst", bufs=1) as cst,
            tc.tile_pool(name="ropeT", bufs=1) as ropeT_pool,
            tc.tile_pool(name="vall", bufs=1) as vall_pool,
        ):
            # q^T and k^T after RoPE: [128, 8, 2048]
            ropeT_sb = ropeT_pool.tile([P, 2 * HPC, S], BF16)
            # V tiles, transposed to [s, d] per 128x128 tile: [128, 64, 128]
            v_all_sb = vall_pool.tile([P, HPC * KT, P], BF16)

            # ---------------- Phase 1: QKV projection + RoPE -------------
            with (
                tc.tile_pool(name="xt", bufs=2) as xt_pool,
                tc.tile_pool(name="wq", bufs=5) as w_pool,
                tc.tile_pool(name="cs", bufs=2) as cs_pool,
                tc.tile_pool(name="qkc", bufs=3) as qkc_pool,
                tc.tile_pool(name="rtmp", bufs=2) as rtmp_pool,
                tc.tile_pool(name="ps_qkv", bufs=3, space="PSUM") as ps_qkv,
                tc.tile_pool(name="ps_rot", bufs=2, space="PSUM") as ps_rot,
                tc.tile_pool(name="ps_vtr", bufs=2, space="PSUM") as ps_vtr,
            ):
                # Pre-issue the b=0 input DMAs (chunked) so the first matmuls
                # start as early as possible; constants go afterwards.
                xt_tiles = {}
                w_tiles = {}
                xt_sb0 = xt_pool.tile([P, ET, SBLK], BF16, tag="xt")
                w_sb0 = w_pool.tile([P, ET, P], BF16, tag="w")
                for ch in range(8):
                    nc.sync.dma_start(w_sb0[:, ts(ch, ET // 8), :],
                                      wt_t[:, ts(ch, ET // 8), ts(0, P)])
                    nc.sync.dma_start(xt_sb0[:, ts(ch, ET // 8), :],
                                      xt_t[:, ts(ch, ET // 8), ts(0, SBLK)])
                xt_tiles[0] = xt_sb0
                w_tiles[(0, 0)] = w_sb0

                # constants (needed later than the first matmuls)
                rt_sb = cst.tile([D, D], BF16)
                nc.sync.dma_start(rt_sb[:], rt_ext.ap())
                ones_sb = cst.tile([P, P], BF16)
                nc.sync.dma_start(ones_sb[:], ones_ext.ap())
                ident_sb = cst.tile([P, P], BF16)
                nc.sync.dma_start(ident_sb[:], ident_ext.ap())
                masks_sb = cst.tile([P, 4, SBLK], BF16)
                nc.sync.dma_start(masks_sb[:], masks_t)

                for b in range(NSBLK):
                    sblk = ts(b, SBLK)
                    xt_sb = xt_tiles.pop(b)
                    cos_sb = cs_pool.tile([D, SBLK], F32, tag="cos")
                    nc.sync.dma_start(cos_sb[:], cost_ext.ap()[:, sblk])
                    sin_sb = cs_pool.tile([D, SBLK], F32, tag="sin")
                    nc.sync.dma_start(sin_sb[:], sins_ext.ap()[:, sblk])

                    for f in range(NF):
                        if f == 3 and b + 1 < NSBLK:
                            # prefetch the next seq block early so its first
                            # matmuls don't wait at the block boundary
                            nxt = xt_pool.tile([P, ET, SBLK], BF16, tag="xt")
                            for ch in range(4):
                                nc.sync.dma_start(
                                    nxt[:, ts(ch, ET // 4), :],
                                    xt_t[:, ts(ch, ET // 4), ts(b + 1, SBLK)])
                            xt_tiles[b + 1] = nxt
                        if (b, f) in w_tiles:
                            w_sb = w_tiles[(b, f)]
                        else:
                            w_sb = w_pool.tile([P, ET, P], BF16, tag="w")
                            nc.sync.dma_start(w_sb[:], wt_t[:, :, ts(f, P)])
                        acc_ps = ps_qkv.tile([P, SBLK], F32, tag="qkv")
                        for e in range(ET):
                            nc.tensor.matmul(
                                acc_ps[:], w_sb[:, e, :], xt_sb[:, e, :],
                                start=(e == 0), stop=(e == ET - 1),
                            )
                        if f < 2 * HPC:
                            # q/k: RoPE.  rotate-half = PE matmul with the
                            # swap matrix (sign lives in the signed-sin
                            # table).  A DMA-based partition swap is cheaper
                            # on paper but head-of-line-blocks the DMA and
                            # DVE queues, which costs far more than 512 PE
                            # columns.
                            qk_sb = qkc_pool.tile([P, SBLK], BF16, tag="qkc")
                            nc.any.tensor_copy(qk_sb[:], acc_ps[:])
                            rot_ps = ps_rot.tile([P, SBLK], F32, tag="rot")
                            nc.tensor.matmul(rot_ps[:], rt_sb[:], qk_sb[:],
                                             start=True, stop=True)
                            t1 = rtmp_pool.tile([P, SBLK], F32, tag="t1")
                            nc.vector.tensor_mul(out=t1[:], in0=acc_ps[:],
                                                 in1=cos_sb[:])
                            t2 = rtmp_pool.tile([P, SBLK], F32, tag="t2")
                            nc.vector.tensor_mul(out=t2[:], in0=rot_ps[:],
                                                 in1=sin_sb[:])
                            nc.vector.tensor_add(
                                out=ropeT_sb[:, f, sblk], in0=t1[:], in1=t2[:])
                        else:
                            # v: bf16 copy, then transpose the four 128x128
                            # tiles inline (keeps the PE warm vs. a separate
                            # transpose block between phases)
                            h = f - 2 * HPC
                            v_sb = qkc_pool.tile([P, SBLK], BF16, tag="vst")
                            nc.any.tensor_copy(v_sb[:], acc_ps[:])
                            for t in range(4):
                                vt_ps = ps_vtr.tile([P, P], BF16, tag="vtr")
                                nc.tensor.transpose(vt_ps[:],
                                                    v_sb[:, ts(t, P)],
                                                    ident_sb[:])
                                nc.any.tensor_copy(
                                    v_all_sb[:, h * KT + 4 * b + t, :],
                                    vt_ps[:])

            # --- Phase 2+3: attention per head, with the previous head's
            # o_proj pass interleaved at matmul granularity.  The o_proj
            # matmuls are pure-PE work that hides the attention phase's
            # Scalar (exp) and Vector (denominator) load; without the
            # interleave, attention is Scalar/DVE-bound and PE idles ~40%.
            with (
                tc.tile_pool(name="wot", bufs=1) as wot_pool,
                tc.tile_pool(name="attnT", bufs=1) as attnT_pool,
                tc.tile_pool(name="exp", bufs=8) as exp_pool,
                tc.tile_pool(name="dsum", bufs=3) as dsum_pool,
                tc.tile_pool(name="dbf", bufs=2) as dbf_pool,
                tc.tile_pool(name="rcp", bufs=2) as rcp_pool,
                tc.tile_pool(name="at", bufs=4) as at_pool,
                tc.tile_pool(name="stage", bufs=4) as stage_pool,
                tc.tile_pool(name="osb", bufs=3) as osb_pool,
                tc.tile_pool(name="part", bufs=1) as part_pool,
                tc.tile_pool(name="ps_sc", bufs=3, space="PSUM") as ps_sc,
                tc.tile_pool(name="ps_av", bufs=2, space="PSUM") as ps_av,
                tc.tile_pool(name="ps_den", bufs=1, space="PSUM") as ps_den,
                tc.tile_pool(name="ps_out", bufs=2, space="PSUM") as ps_out,
            ):
                wot_sb = wot_pool.tile([P, ET, ECOLS], BF16)
                nc.sync.dma_start(wot_sb[:], wot_t)
                attnT_sb = attnT_pool.tile([P, HPC, S], BF16)
                NST = S // P
                part_sb = part_pool.tile([P, NST, ECOLS], F32)

                def op_tile(p_h, st):
                    # one o_proj seq-tile of pass p_h: [128 seq, 512 e-cols],
                    # contracting the 8 128-row blocks of ccout[p_h][st//4].
                    # PSUM is evacuated by the Scalar engine into an SBUF
                    # stage so the PE never waits on the (busy) Vector
                    # engine; the part-sum add runs on DVE off the critical
                    # path.
                    a_sb = at_pool.tile([P, 8, P], BF16, tag="at")
                    nc.sync.dma_start(
                        a_sb[:], ccout_ts[p_h][st // 4][:, :, ts(st % 4, P)])
                    o_ps = ps_out.tile([P, ECOLS], F32, tag="out")
                    for c in range(8):
                        nc.tensor.matmul(o_ps[:], a_sb[:, c, :],
                                         wot_sb[:, 4 * c + p_h, :],
                                         start=(c == 0), stop=(c == 7))
                    if p_h == 0:
                        nc.scalar.copy(part_sb[:, st, :], o_ps[:])
                    else:
                        stg = stage_pool.tile([P, ECOLS], F32, tag="stg")
                        nc.scalar.copy(stg[:], o_ps[:])
                        if p_h < HPC - 1:
                            nc.vector.tensor_add(out=part_sb[:, st, :],
                                                 in0=part_sb[:, st, :],
                                                 in1=stg[:])
                        else:
                            o_sb = osb_pool.tile([P, ECOLS], F32, tag="osb")
                            nc.vector.tensor_add(out=o_sb[:], in0=stg[:],
                                                 in1=part_sb[:, st, :])
                            # gpsimd queue: this DMA waits on the DVE add,
                            # and must not block later a_sb loads on sync
                            nc.gpsimd.dma_start(out_ext.ap()[ts(st, P), :],
                                                o_sb[:])

                # attention head h has 40 score tiles; the 16 o_proj tiles
                # of pass h-1 slot in every other tile from tile 10 on (the
                # per-block AllGathers of head h-1 have all landed by then).
                OP_START = 10

                for h in range(HPC):
                    v_sb = v_all_sb[:, h * KT:(h + 1) * KT, :]
                    qh = ropeT_sb[:, h, :]
                    kh = ropeT_sb[:, HPC + h, :]
                    op_queue = [(h - 1, st) for st in range(NST)] if h else []
                    icount = 0
                    for j in range(NSBLK):
                        nkt = 4 * j + 4
                        av_ps = ps_av.tile([P, SBLK], F32, tag="av")
                        dsum = dsum_pool.tile([P, SBLK], F32, tag="dsum")
                        for i in range(nkt):
                            # diagonal tile r: columns below 128r are
                            # fully masked -> compute only [off:SBLK]
                            r = i - 4 * j
                            off = 128 * r if r > 0 else 0
                            qs = bass.ds(j * SBLK + off, SBLK - off)
                            sc_ps = ps_sc.tile([P, SBLK], F32, tag="sc")
                            nc.tensor.matmul(sc_ps[:, off:], kh[:, ts(i, P)],
                                             qh[:, qs],
                                             start=True, stop=True)
                            exp_sb = exp_pool.tile([P, SBLK], BF16,
                                                   tag="exp")
                            nc.scalar.activation(
                                exp_sb[:, off:], sc_ps[:, off:],
                                mybir.ActivationFunctionType.Exp,
                                scale=SCALE)
                            if r >= 0:
                                nc.vector.tensor_mul(
                                    out=exp_sb[:, off:],
                                    in0=exp_sb[:, off:],
                                    in1=masks_sb[:, r, off:])
                            # denominator partial sums accumulate on DVE
                            # (fp32); one matmul per (h, j) replicates the
                            # k-sum across partitions afterwards.
                            if i == 0:
                                nc.vector.tensor_copy(dsum[:], exp_sb[:])
                            else:
                                nc.vector.tensor_add(
                                    out=dsum[:, off:], in0=dsum[:, off:],
                                    in1=exp_sb[:, off:])
                            nc.tensor.matmul(
                                av_ps[:, off:], v_sb[:, i, :],
                                exp_sb[:, off:],
                                start=(i == 0), stop=(i == nkt - 1))
                            icount += 1
                            if (op_queue and icount >= OP_START
                                    and icount % 2 == 0):
                                op_tile(*op_queue.pop(0))
                        dbf = dbf_pool.tile([P, SBLK], BF16, tag="dbf")
                        nc.vector.tensor_copy(dbf[:], dsum[:])
                        den_ps = ps_den.tile([P, SBLK], F32, tag="den")
                        nc.tensor.matmul(den_ps[:], ones_sb[:], dbf[:],
                                         start=True, stop=True)
                        recip_sb = rcp_pool.tile([P, SBLK], F32,
                                                 tag="rcp")
                        nc.vector.reciprocal_approx_fast(
                            out=recip_sb[:], in_=den_ps[:])
                        nc.vector.tensor_mul(
                            out=attnT_sb[:, h, ts(j, SBLK)],
                            in0=av_ps[:], in1=recip_sb[:])
                        # ship this q-block immediately: the chunked
                        # AllGather overlaps the rest of the head and lets
                        # o_proj consume early blocks of the final head
                        # without waiting for the whole head to finish.
                        nc.gpsimd.dma_start(
                            ccins[h][j].ap().rearrange("(o p) s -> p o s",
                                                       p=P),
                            attnT_sb[:, h:h + 1, ts(j, SBLK)])
                        nc.gpsimd.collective_compute(
                            "AllGather", mybir.AluOpType.bypass,
                            replica_groups=[list(range(NCORES))],
                            ins=[ccins[h][j].ap()],
                            outs=[ccouts[h][j].ap()],
                        )

                    while op_queue:
                        op_tile(*op_queue.pop(0))

                # final o_proj pass (local head 3) after its AllGathers
                for st in range(NST):
                    op_tile(HPC - 1, st)

    nc.compile()
    _NC_CACHE = nc
    return nc


def _prep_inputs(hidden_states, cos, sin, w_pack, w_o):
    hs = np.asarray(hidden_states, dtype=np.float32).reshape(S, E)
    xt = np.ascontiguousarray(hs.T).astype(BF16NP)
    cost = np.ascontiguousarray(np.asarray(cos, dtype=np.float32).T)
    sint = np.ascontiguousarray(np.asarray(sin, dtype=np.float32).T)
    # signed sin table: rotate_half's sign folded in (rows 0..63 negated)
    sins = sint.copy()
    sins[:HALF] = -sins[:HALF]
    w_pack = np.asarray(w_pack, dtype=np.float32)
    w_o = np.asarray(w_o, dtype=np.float32)

    # rotate-half as a matmul: plain half-swap (sign lives in sins)
    R = np.zeros((D, D), dtype=np.float32)
    for dp in range(HALF):
        R[dp, dp + HALF] = 1.0
        R[dp + HALF, dp] = 1.0
    rt = np.ascontiguousarray(R.T).astype(BF16NP)

    masks = np.zeros((4, P, SBLK), dtype=np.float32)
    kk = np.arange(P)[:, None]
    qq = np.arange(SBLK)[None, :]
    for r in range(4):
        masks[r] = (P * r + kk <= qq).astype(np.float32)
    masks = masks.astype(BF16NP)

    ones = np.ones((P, P), dtype=BF16NP)
    ident = np.eye(P, dtype=np.float32).astype(BF16NP)

    in_maps = []
    hw = E // NCORES  # 512 head-rows per core in each of q/k/v
    for c in range(NCORES):
        rows = slice(c * hw, (c + 1) * hw)
        wqkv = np.concatenate(
            [w_pack[rows], w_pack[E:][rows], w_pack[2 * E:][rows]], axis=0)
        wt = np.ascontiguousarray(wqkv.T).astype(BF16NP)
        wot = np.ascontiguousarray(w_o[rows].T).astype(BF16NP)
        in_maps.append({
            "xt": xt, "wt": wt, "wot": wot,
            "cost": cost, "sins": sins, "rt": rt,
            "masks": masks, "ones": ones, "ident": ident,
        })
    return in_maps


def run(trace=False, trace_cores=None, **inputs):
    nc = build()
    in_maps = _prep_inputs(**inputs)
    res = run_bass_kernel_spmd(
        nc, in_maps, core_ids=list(range(NCORES)),
        trace=trace, trace_cores=trace_cores,
    )
    out = np.concatenate([res.results[c]["out"] for c in range(NCORES)], axis=1)
    return out.reshape(B, S, E).astype(np.float32), res


def kernel(**inputs) -> np.ndarray:
    out, _ = run(trace=False, **inputs)
    return out


# revision 27
# speedup vs baseline: 1.0400x; 1.0001x over previous
"""Baichuan attention layer (B=1, S=2048, E=4096, H=32, D=128) on 8 Trainium2
NeuronCores.

Sharding:
- QKV projection + RoPE + causal attention: tensor-parallel by head (4 heads
  per core). All per-head tensors live in transposed [feature, seq] layout so
  every matmul contracts over the partition dim with zero transposes:
    qkv^T[f, s]   = W @ X^T                (lhsT = W^T tiles, rhs = X^T tiles)
    scores^T[k,q] = K @ Q^T                (lhsT = K^T tile, rhs = Q^T block)
    att^T[d, q]   = V^T @ P^T              (lhsT = V tile,   rhs = exp tile)
  Softmax runs without max-subtraction (scores ~ N(0,1) after 1/sqrt(D), fp32
  exp is safe).  The denominator is accumulated on the Vector engine (exp
  tiles summed elementwise in fp32), then one all-ones [128,128] lhsT matmul
  per (head, q-block) replicates the k-sum across all PSUM partitions.
  RoPE's rotate-half is a partition swap done by an SBUF->SBUF DMA, with the
  sign folded into a host-precomputed signed-sin table; no PE matmul needed.
- One AllGather of att^T [512, 2048] bf16 per core -> full att^T [4096, 2048].
- o_proj: column-parallel (each core computes its 512 output columns for the
  full sequence, using its slice of w_o). Host concatenates along E.

All matmuls in bf16 with fp32 PSUM accumulation.
"""

import importlib.util
import sys
import types

import numpy as np
import ml_dtypes

BF16NP = ml_dtypes.bfloat16

B, S, E = 1, 2048, 4096
H, D = 32, 128
NCORES = 8
HPC = H // NCORES          # heads per core = 4
P = 128                    # partitions
SBLK = 512                 # seq block (matmul free dim)
NSBLK = S // SBLK          # 4
ET = E // P                # 32 e-tiles
NF = 3 * HPC               # 12 f-tiles per core (q0..3, k4..7, v8..11)
KT = S // P                # 16 k-tiles
ECOLS = E // NCORES        # 512 output columns per core
SCALE = 1.0 / float(np.sqrt(D))
HALF = D // 2


def _install_ntff_hook():
    """antenv.axon_hooks is absent in this image; recreate it from trn_boot's
    ctypes shim so run_bass_kernel_spmd(trace=True) can capture NTFF traces."""
    if "antenv.axon_hooks" in sys.modules:
        return
    try:
        spec = importlib.util.spec_from_file_location(
            "trn_boot", "/root/.axon_site/trn_agent_boot/trn_boot.py")
        tb = importlib.util.module_from_spec(spec)
        spec.loader.exec_module(tb)
        hook = tb._ntff_profile_via_ctypes("/opt/axon/libaxon_pjrt.so")
    except Exception:
        hook = None
    mod = types.ModuleType("antenv.axon_hooks")
    mod.get_axon_ntff_profile_hook = lambda: hook
    mod.set_axon_ntff_profile_hook = lambda h: None
    sys.modules["antenv.axon_hooks"] = mod


_install_ntff_hook()

import concourse.bass as bass  # noqa: E402
import concourse.mybir as mybir  # noqa: E402
import concourse.tile as tile  # noqa: E402
from concourse import bacc  # noqa: E402
from concourse.bass import ts  # noqa: E402
from concourse.bass_utils import run_bass_kernel_spmd  # noqa: E402

BF16 = mybir.dt.bfloat16
F32 = mybir.dt.float32

_NC_CACHE = None


def build():
    global _NC_CACHE
    if _NC_CACHE is not None:
        return _NC_CACHE
    nc = bacc.Bacc("TRN2", target_bir_lowering=False, debug=False,
                   num_devices=NCORES)

    xt_ext = nc.dram_tensor("xt", [E, S], BF16, kind="ExternalInput")
    wt_ext = nc.dram_tensor("wt", [E, NF * P], BF16, kind="ExternalInput")
    wot_ext = nc.dram_tensor("wot", [E, ECOLS], BF16, kind="ExternalInput")
    cost_ext = nc.dram_tensor("cost", [D, S], F32, kind="ExternalInput")
    sins_ext = nc.dram_tensor("sins", [D, S], F32, kind="ExternalInput")
    rt_ext = nc.dram_tensor("rt", [D, D], BF16, kind="ExternalInput")
    masks_ext = nc.dram_tensor("masks", [4, P, SBLK], BF16, kind="ExternalInput")
    ones_ext = nc.dram_tensor("ones", [P, P], BF16, kind="ExternalInput")
    ident_ext = nc.dram_tensor("ident", [P, P], BF16, kind="ExternalInput")
    out_ext = nc.dram_tensor("out", [S, ECOLS], F32, kind="ExternalOutput")

    # One AllGather per (local head, q-block) chunk, issued as soon as that
    # block's attention output is ready: they overlap attention/o_proj
    # compute, and the fine granularity lets o_proj start on a head's early
    # q-blocks while later blocks are still in flight.  ccout[h][j] rank-r
    # block = rows [128r, 128r+128) = global head 4r + h, seq block j.
    ccins = [[nc.dram_tensor(f"ccin{h}_{j}", [P, SBLK], BF16)
              for j in range(NSBLK)] for h in range(HPC)]
    ccouts = [[nc.dram_tensor(f"ccout{h}_{j}", [NCORES * P, SBLK], BF16,
                              addr_space="Shared") for j in range(NSBLK)]
              for h in range(HPC)]

    xt_t = xt_ext.ap().rearrange("(eo p) s -> p eo s", p=P)
    wt_t = wt_ext.ap().rearrange("(eo p) f -> p eo f", p=P)
    wot_t = wot_ext.ap().rearrange("(fo p) e -> p fo e", p=P)
    masks_t = masks_ext.ap().rearrange("r p q -> p r q")
    # [p, c, s]: block c of ccout[h][j] = global head 4c + h
    ccout_ts = [[cc.ap().rearrange("(c p) s -> p c s", p=P) for cc in row]
                for row in ccouts]

    with tile.TileContext(nc) as tc:
        with (
            tc.tile_pool(name="cst", bufs=1) as cst,
            tc.tile_pool(name="ropeT", bufs=1) as ropeT_pool,
            tc.tile_pool(name="vall", bufs=1) as vall_pool,
        ):
            # q^T and k^T after RoPE: [128, 8, 2048]
            ropeT_sb = ropeT_pool.tile([P, 2 * HPC, S], BF16)
            # V tiles, transposed to [s, d] per 128x128 tile: [128, 64, 128]
            v_all_sb = vall_pool.tile([P, HPC * KT, P], BF16)

            # ---------------- Phase 1: QKV projection + RoPE -------------
            with (
                tc.tile_pool(name="xt", bufs=2) as xt_pool,
                tc.tile_pool(name="wq", bufs=5) as w_pool,
                tc.tile_pool(name="cs", bufs=2) as cs_pool,
                tc.tile_pool(name="qkc", bufs=3) as qkc_pool,
                tc.tile_pool(name="rtmp", bufs=2) as rtmp_pool,
                tc.tile_pool(name="ps_qkv", bufs=3, space="PSUM") as ps_qkv,
                tc.tile_pool(name="ps_rot", bufs=2, space="PSUM") as ps_rot,
                tc.tile_pool(name="ps_vtr", bufs=2, space="PSUM") as ps_vtr,
            ):
                # Pre-issue the b=0 input DMAs (chunked) so the first matmuls
                # start as early as possible; constants go afterwards.
                xt_tiles = {}
                w_tiles = {}
                xt_sb0 = xt_pool.tile([P, ET, SBLK], BF16, tag="xt")
                w_sb0 = w_pool.tile([P, ET, P], BF16, tag="w")
                for ch in range(8):
                    nc.sync.dma_start(w_sb0[:, ts(ch, ET // 8), :],
                                      wt_t[:, ts(ch, ET // 8), ts(0, P)])
                    nc.sync.dma_start(xt_sb0[:, ts(ch, ET // 8), :],
                                      xt_t[:, ts(ch, ET // 8), ts(0, SBLK)])
                xt_tiles[0] = xt_sb0
                w_tiles[(0, 0)] = w_sb0

                # constants (needed later than the first matmuls), issued on
                # the vector queue so the sync stream stays a pure xt/w feed
                rt_sb = cst.tile([D, D], BF16)
                nc.scalar.dma_start(rt_sb[:], rt_ext.ap())
                ones_sb = cst.tile([P, P], BF16)
                nc.scalar.dma_start(ones_sb[:], ones_ext.ap())
                ident_sb = cst.tile([P, P], BF16)
                nc.scalar.dma_start(ident_sb[:], ident_ext.ap())
                masks_sb = cst.tile([P, 4, SBLK], BF16)
                nc.scalar.dma_start(masks_sb[:], masks_t)

                for b in range(NSBLK):
                    sblk = ts(b, SBLK)
                    xt_sb = xt_tiles.pop(b)
                    cos_sb = cs_pool.tile([D, SBLK], F32, tag="cos")
                    nc.scalar.dma_start(cos_sb[:], cost_ext.ap()[:, sblk])
                    sin_sb = cs_pool.tile([D, SBLK], F32, tag="sin")
                    nc.scalar.dma_start(sin_sb[:], sins_ext.ap()[:, sblk])

                    for f in range(NF):
                        if f == 3 and b + 1 < NSBLK:
                            # prefetch the next seq block early so its first
                            # matmuls don't wait at the block boundary
                            nxt = xt_pool.tile([P, ET, SBLK], BF16, tag="xt")
                            for ch in range(4):
                                nc.sync.dma_start(
                                    nxt[:, ts(ch, ET // 4), :],
                                    xt_t[:, ts(ch, ET // 4), ts(b + 1, SBLK)])
                            xt_tiles[b + 1] = nxt
                        if (b, f) in w_tiles:
                            w_sb = w_tiles[(b, f)]
                        else:
                            w_sb = w_pool.tile([P, ET, P], BF16, tag="w")
                            nc.sync.dma_start(w_sb[:], wt_t[:, :, ts(f, P)])
                        acc_ps = ps_qkv.tile([P, SBLK], F32, tag="qkv")
                        for e in range(ET):
                            nc.tensor.matmul(
                                acc_ps[:], w_sb[:, e, :], xt_sb[:, e, :],
                                start=(e == 0), stop=(e == ET - 1),
                            )
                        if f < 2 * HPC:
                            # q/k: RoPE.  rotate-half = PE matmul with the
                            # swap matrix (sign lives in the signed-sin
                            # table).  A DMA-based partition swap is cheaper
                            # on paper but head-of-line-blocks the DMA and
                            # DVE queues, which costs far more than 512 PE
                            # columns.
                            qk_sb = qkc_pool.tile([P, SBLK], BF16, tag="qkc")
                            nc.any.tensor_copy(qk_sb[:], acc_ps[:])
                            rot_ps = ps_rot.tile([P, SBLK], F32, tag="rot")
                            nc.tensor.matmul(rot_ps[:], rt_sb[:], qk_sb[:],
                                             start=True, stop=True)
                            t1 = rtmp_pool.tile([P, SBLK], F32, tag="t1")
                            nc.vector.tensor_mul(out=t1[:], in0=acc_ps[:],
                                                 in1=cos_sb[:])
                            t2 = rtmp_pool.tile([P, SBLK], F32, tag="t2")
                            nc.vector.tensor_mul(out=t2[:], in0=rot_ps[:],
                                                 in1=sin_sb[:])
                            nc.vector.tensor_add(
                                out=ropeT_sb[:, f, sblk], in0=t1[:], in1=t2[:])
                        else:
                            # v: bf16 copy, then transpose the four 128x128
                            # tiles inline (keeps the PE warm vs. a separate
                            # transpose block between phases)
                            h = f - 2 * HPC
                            v_sb = qkc_pool.tile([P, SBLK], BF16, tag="vst")
                            nc.any.tensor_copy(v_sb[:], acc_ps[:])
                            for t in range(4):
                                vt_ps = ps_vtr.tile([P, P], BF16, tag="vtr")
                                nc.tensor.transpose(vt_ps[:],
                                                    v_sb[:, ts(t, P)],
                                                    ident_sb[:])
                                nc.any.tensor_copy(
                                    v_all_sb[:, h * KT + 4 * b + t, :],
                                    vt_ps[:])

            # --- Phase 2+3: attention per head, with the previous head's
            # o_proj pass interleaved at matmul granularity.  The o_proj
            # matmuls are pure-PE work that hides the attention phase's
            # Scalar (exp) and Vector (denominator) load; without the
            # interleave, attention is Scalar/DVE-bound and PE idles ~40%.
            with (
                tc.tile_pool(name="wot", bufs=1) as wot_pool,
                tc.tile_pool(name="attnT", bufs=1) as attnT_pool,
                tc.tile_pool(name="exp", bufs=8) as exp_pool,
                tc.tile_pool(name="dsum", bufs=3) as dsum_pool,
                tc.tile_pool(name="rcp", bufs=2) as rcp_pool,
                tc.tile_pool(name="at", bufs=4) as at_pool,
                tc.tile_pool(name="osb", bufs=3) as osb_pool,
                tc.tile_pool(name="part", bufs=1) as part_pool,
                tc.tile_pool(name="ps_sc", bufs=3, space="PSUM") as ps_sc,
                tc.tile_pool(name="ps_av", bufs=2, space="PSUM") as ps_av,
                tc.tile_pool(name="ps_den", bufs=1, space="PSUM") as ps_den,
                tc.tile_pool(name="ps_out", bufs=2, space="PSUM") as ps_out,
            ):
                wot_sb = wot_pool.tile([P, ET, ECOLS], BF16)
                nc.sync.dma_start(wot_sb[:], wot_t)
                attnT_sb = attnT_pool.tile([P, HPC, S], BF16)
                NST = S // P
                part_sb = part_pool.tile([P, NST, ECOLS], F32)

                def op_tile(p_h, st):
                    # one o_proj seq-tile of pass p_h: [128 seq, 512 e-cols],
                    # contracting the 8 128-row blocks of ccout[p_h][st//4]
                    a_sb = at_pool.tile([P, 8, P], BF16, tag="at")
                    nc.sync.dma_start(
                        a_sb[:], ccout_ts[p_h][st // 4][:, :, ts(st % 4, P)])
                    o_ps = ps_out.tile([P, ECOLS], F32, tag="out")
                    for c in range(8):
                        nc.tensor.matmul(o_ps[:], a_sb[:, c, :],
                                         wot_sb[:, 4 * c + p_h, :],
                                         start=(c == 0), stop=(c == 7))
                    if p_h == 0:
                        nc.vector.tensor_copy(part_sb[:, st, :], o_ps[:])
                    elif p_h < HPC - 1:
                        nc.vector.tensor_add(out=part_sb[:, st, :],
                                             in0=part_sb[:, st, :],
                                             in1=o_ps[:])
                    else:
                        o_sb = osb_pool.tile([P, ECOLS], F32, tag="osb")
                        nc.vector.tensor_add(out=o_sb[:], in0=o_ps[:],
                                             in1=part_sb[:, st, :])
                        # gpsimd queue: must not block a_sb loads on sync
                        nc.gpsimd.dma_start(out_ext.ap()[ts(st, P), :],
                                            o_sb[:])

                # attention head h has 40 score tiles; the 16 o_proj tiles
                # of pass h-1 slot in every other tile from tile 10 on (the
                # per-block AllGathers of head h-1 have all landed by then).
                OP_START = 10

                for h in range(HPC):
                    v_sb = v_all_sb[:, h * KT:(h + 1) * KT, :]
                    qh = ropeT_sb[:, h, :]
                    kh = ropeT_sb[:, HPC + h, :]
                    op_queue = [(h - 1, st) for st in range(NST)] if h else []
                    icount = 0
                    for j in range(NSBLK):
                        nkt = 4 * j + 4
                        av_ps = ps_av.tile([P, SBLK], F32, tag="av")
                        # bf16 accumulation is safe here: each dsum element
                        # sums at most 16 exp tiles (the heavy 2048-wide sum
                        # happens in the fp32-PSUM ones-matmul below), and
                        # bf16 runs the DVE in 2x mode.
                        dsum = dsum_pool.tile([P, SBLK], BF16, tag="dsum")
                        for i in range(nkt):
                            # diagonal tile r: columns below 128r are
                            # fully masked -> compute only [off:SBLK]
                            r = i - 4 * j
                            off = 128 * r if r > 0 else 0
                            qs = bass.ds(j * SBLK + off, SBLK - off)
                            sc_ps = ps_sc.tile([P, SBLK], F32, tag="sc")
                            nc.tensor.matmul(sc_ps[:, off:], kh[:, ts(i, P)],
                                             qh[:, qs],
                                             start=True, stop=True)
                            exp_sb = exp_pool.tile([P, SBLK], BF16,
                                                   tag="exp")
                            nc.scalar.activation(
                                exp_sb[:, off:], sc_ps[:, off:],
                                mybir.ActivationFunctionType.Exp,
                                scale=SCALE)
                            if r >= 0:
                                nc.vector.tensor_mul(
                                    out=exp_sb[:, off:],
                                    in0=exp_sb[:, off:],
                                    in1=masks_sb[:, r, off:])
                            # denominator partial sums accumulate on DVE
                            # (fp32); one matmul per (h, j) replicates the
                            # k-sum across partitions afterwards.
                            if i == 0:
                                nc.vector.tensor_copy(dsum[:], exp_sb[:])
                            else:
                                nc.vector.tensor_add(
                                    out=dsum[:, off:], in0=dsum[:, off:],
                                    in1=exp_sb[:, off:])
                            nc.tensor.matmul(
                                av_ps[:, off:], v_sb[:, i, :],
                                exp_sb[:, off:],
                                start=(i == 0), stop=(i == nkt - 1))
                            icount += 1
                            if (op_queue and icount >= OP_START
                                    and icount % 2 == 0):
                                op_tile(*op_queue.pop(0))
                        den_ps = ps_den.tile([P, SBLK], F32, tag="den")
                        nc.tensor.matmul(den_ps[:], ones_sb[:], dsum[:],
                                         start=True, stop=True)
                        recip_sb = rcp_pool.tile([P, SBLK], F32,
                                                 tag="rcp")
                        nc.vector.reciprocal_approx_fast(
                            out=recip_sb[:], in_=den_ps[:])
                        nc.vector.tensor_mul(
                            out=attnT_sb[:, h, ts(j, SBLK)],
                            in0=av_ps[:], in1=recip_sb[:])
                        # ship this q-block immediately: the chunked
                        # AllGather overlaps the rest of the head and lets
                        # o_proj consume early blocks of the final head
                        # without waiting for the whole head to finish.
                        nc.gpsimd.dma_start(
                            ccins[h][j].ap().rearrange("(o p) s -> p o s",
                                                       p=P),
                            attnT_sb[:, h:h + 1, ts(j, SBLK)])
                        nc.gpsimd.collective_compute(
                            "AllGather", mybir.AluOpType.bypass,
                            replica_groups=[list(range(NCORES))],
                            ins=[ccins[h][j].ap()],
                            outs=[ccouts[h][j].ap()],
                        )

                    while op_queue:
                        op_tile(*op_queue.pop(0))

                # final o_proj pass (local head 3) after its AllGathers
                for st in range(NST):
                    op_tile(HPC - 1, st)

    nc.compile()
    _NC_CACHE = nc
    return nc


def _prep_inputs(hidden_states, cos, sin, w_pack, w_o):
    hs = np.asarray(hidden_states, dtype=np.float32).reshape(S, E)
    xt = np.ascontiguousarray(hs.T).astype(BF16NP)
    cost = np.ascontiguousarray(np.asarray(cos, dtype=np.float32).T)
    sint = np.ascontiguousarray(np.asarray(sin, dtype=np.float32).T)
    # signed sin table: rotate_half's sign folded in (rows 0..63 negated)
    sins = sint.copy()
    sins[:HALF] = -sins[:HALF]
    w_pack = np.asarray(w_pack, dtype=np.float32)
    w_o = np.asarray(w_o, dtype=np.float32)

    # rotate-half as a matmul: plain half-swap (sign lives in sins)
    R = np.zeros((D, D), dtype=np.float32)
    for dp in range(HALF):
        R[dp, dp + HALF] = 1.0
        R[dp + HALF, dp] = 1.0
    rt = np.ascontiguousarray(R.T).astype(BF16NP)

    masks = np.zeros((4, P, SBLK), dtype=np.float32)
    kk = np.arange(P)[:, None]
    qq = np.arange(SBLK)[None, :]
    for r in range(4):
        masks[r] = (P * r + kk <= qq).astype(np.float32)
    masks = masks.astype(BF16NP)

    ones = np.ones((P, P), dtype=BF16NP)
    ident = np.eye(P, dtype=np.float32).astype(BF16NP)

    in_maps = []
    hw = E // NCORES  # 512 head-rows per core in each of q/k/v
    for c in range(NCORES):
        rows = slice(c * hw, (c + 1) * hw)
        wqkv = np.concatenate(
            [w_pack[rows], w_pack[E:][rows], w_pack[2 * E:][rows]], axis=0)
        wt = np.ascontiguousarray(wqkv.T).astype(BF16NP)
        wot = np.ascontiguousarray(w_o[rows].T).astype(BF16NP)
        in_maps.append({
            "xt": xt, "wt": wt, "wot": wot,
            "cost": cost, "sins": sins, "rt": rt,
            "masks": masks, "ones": ones, "ident": ident,
        })
    return in_maps


def run(trace=False, trace_cores=None, **inputs):
    nc = build()
    in_maps = _prep_inputs(**inputs)
    res = run_bass_kernel_spmd(
        nc, in_maps, core_ids=list(range(NCORES)),
        trace=trace, trace_cores=trace_cores,
    )
    out = np.concatenate([res.results[c]["out"] for c in range(NCORES)], axis=1)
    return out.reshape(B, S, E).astype(np.float32), res


def kernel(**inputs) -> np.ndarray:
    out, _ = run(trace=False, **inputs)
    return out


# revision 31
# speedup vs baseline: 1.1087x; 1.0660x over previous
"""Baichuan attention layer (B=1, S=2048, E=4096, H=32, D=128) on 8 Trainium2
NeuronCores.

Sharding:
- QKV projection + RoPE + causal attention: tensor-parallel by head (4 heads
  per core). All per-head tensors live in transposed [feature, seq] layout so
  every matmul contracts over the partition dim with zero transposes:
    qkv^T[f, s]   = W @ X^T                (lhsT = W^T tiles, rhs = X^T tiles)
    scores^T[k,q] = K @ Q^T                (lhsT = K^T tile, rhs = Q^T block)
    att^T[d, q]   = V^T @ P^T              (lhsT = V tile,   rhs = exp tile)
  Softmax runs without max-subtraction (scores ~ N(0,1) after 1/sqrt(D), fp32
  exp is safe).  The denominator is accumulated on the Vector engine (exp
  tiles summed elementwise in fp32), then one all-ones [128,128] lhsT matmul
  per (head, q-block) replicates the k-sum across all PSUM partitions.
  RoPE's rotate-half is a partition swap done by an SBUF->SBUF DMA, with the
  sign folded into a host-precomputed signed-sin table; no PE matmul needed.
- One AllGather of att^T [512, 2048] bf16 per core -> full att^T [4096, 2048].
- o_proj: column-parallel (each core computes its 512 output columns for the
  full sequence, using its slice of w_o). Host concatenates along E.

All matmuls in bf16 with fp32 PSUM accumulation.
"""

import importlib.util
import sys
import types

import numpy as np
import ml_dtypes

BF16NP = ml_dtypes.bfloat16

B, S, E = 1, 2048, 4096
H, D = 32, 128
NCORES = 8
HPC = H // NCORES          # heads per core = 4
P = 128                    # partitions
SBLK = 512                 # seq block (matmul free dim)
NSBLK = S // SBLK          # 4
ET = E // P                # 32 e-tiles
NF = 3 * HPC               # 12 f-tiles per core (q0..3, k4..7, v8..11)
KT = S // P                # 16 k-tiles
ECOLS = E // NCORES        # 512 output columns per core
SCALE = 1.0 / float(np.sqrt(D))
HALF = D // 2


def _install_ntff_hook():
    """antenv.axon_hooks is absent in this image; recreate it from trn_boot's
    ctypes shim so run_bass_kernel_spmd(trace=True) can capture NTFF traces."""
    if "antenv.axon_hooks" in sys.modules:
        return
    try:
        spec = importlib.util.spec_from_file_location(
            "trn_boot", "/root/.axon_site/trn_agent_boot/trn_boot.py")
        tb = importlib.util.module_from_spec(spec)
        spec.loader.exec_module(tb)
        hook = tb._ntff_profile_via_ctypes("/opt/axon/libaxon_pjrt.so")
    except Exception:
        hook = None
    mod = types.ModuleType("antenv.axon_hooks")
    mod.get_axon_ntff_profile_hook = lambda: hook
    mod.set_axon_ntff_profile_hook = lambda h: None
    sys.modules["antenv.axon_hooks"] = mod


_install_ntff_hook()

import concourse.bass as bass  # noqa: E402
import concourse.mybir as mybir  # noqa: E402
import concourse.tile as tile  # noqa: E402
from concourse import bacc  # noqa: E402
from concourse.bass import ts  # noqa: E402
from concourse.bass_utils import run_bass_kernel_spmd  # noqa: E402

BF16 = mybir.dt.bfloat16
F32 = mybir.dt.float32

_NC_CACHE = None


def build():
    global _NC_CACHE
    if _NC_CACHE is not None:
        return _NC_CACHE
    nc = bacc.Bacc("TRN2", target_bir_lowering=False, debug=False,
                   num_devices=NCORES)

    xt_ext = nc.dram_tensor("xt", [E, S], BF16, kind="ExternalInput")
    wt_ext = nc.dram_tensor("wt", [E, NF * P], BF16, kind="ExternalInput")
    wot_ext = nc.dram_tensor("wot", [E, ECOLS], BF16, kind="ExternalInput")
    cost_ext = nc.dram_tensor("cost", [D, S], F32, kind="ExternalInput")
    sins_ext = nc.dram_tensor("sins", [D, S], F32, kind="ExternalInput")
    rt_ext = nc.dram_tensor("rt", [D, D], BF16, kind="ExternalInput")
    masks_ext = nc.dram_tensor("masks", [4, P, SBLK], BF16, kind="ExternalInput")
    ones_ext = nc.dram_tensor("ones", [P, P], BF16, kind="ExternalInput")
    ident_ext = nc.dram_tensor("ident", [P, P], BF16, kind="ExternalInput")
    out_ext = nc.dram_tensor("out", [S, ECOLS], F32, kind="ExternalOutput")

    # Two AllGathers per local head (seq halves), issued as soon as each
    # half's attention output is ready: they overlap attention/o_proj
    # compute, and the split lets o_proj consume the final head's first
    # half while its second half is still in flight.  Each collective has
    # ~15us of fixed cost on the serial CC stream, so fewer+bigger wins;
    # a dummy warmup AllGather during QKV absorbs the ~40us cold start.
    # ccout[h][c] rank-r block = rows [128r, 128r+128) = global head 4r+h.
    NCH = 2
    SCH = S // NCH
    ccins = [[nc.dram_tensor(f"ccin{h}_{c}", [P, SCH], BF16)
              for c in range(NCH)] for h in range(HPC)]
    ccouts = [[nc.dram_tensor(f"ccout{h}_{c}", [NCORES * P, SCH], BF16,
                              addr_space="Shared") for c in range(NCH)]
              for h in range(HPC)]
    warm_in = nc.dram_tensor("warmin", [P, P], BF16)
    warm_out = nc.dram_tensor("warmout", [NCORES * P, P], BF16,
                              addr_space="Shared")

    xt_t = xt_ext.ap().rearrange("(eo p) s -> p eo s", p=P)
    wt_t = wt_ext.ap().rearrange("(eo p) f -> p eo f", p=P)
    wot_t = wot_ext.ap().rearrange("(fo p) e -> p fo e", p=P)
    masks_t = masks_ext.ap().rearrange("r p q -> p r q")
    # [p, c, s]: block c of ccout[h][j] = global head 4c + h
    ccout_ts = [[cc.ap().rearrange("(c p) s -> p c s", p=P) for cc in row]
                for row in ccouts]

    with tile.TileContext(nc) as tc:
        with (
            tc.tile_pool(name="cst", bufs=1) as cst,
            tc.tile_pool(name="ropeT", bufs=1) as ropeT_pool,
            tc.tile_pool(name="vall", bufs=1) as vall_pool,
        ):
            # q^T and k^T after RoPE: [128, 8, 2048]
            ropeT_sb = ropeT_pool.tile([P, 2 * HPC, S], BF16)
            # V tiles, transposed to [s, d] per 128x128 tile: [128, 64, 128]
            v_all_sb = vall_pool.tile([P, HPC * KT, P], BF16)

            # ---------------- Phase 1: QKV projection + RoPE -------------
            with (
                tc.tile_pool(name="xt", bufs=2) as xt_pool,
                tc.tile_pool(name="wq", bufs=5) as w_pool,
                tc.tile_pool(name="cs", bufs=2) as cs_pool,
                tc.tile_pool(name="qkc", bufs=3) as qkc_pool,
                tc.tile_pool(name="rtmp", bufs=2) as rtmp_pool,
                tc.tile_pool(name="ps_qkv", bufs=3, space="PSUM") as ps_qkv,
                tc.tile_pool(name="ps_rot", bufs=2, space="PSUM") as ps_rot,
                tc.tile_pool(name="ps_vtr", bufs=2, space="PSUM") as ps_vtr,
            ):
                # Pre-issue the b=0 input DMAs (chunked) so the first matmuls
                # start as early as possible; constants go afterwards.
                xt_tiles = {}
                w_tiles = {}
                xt_sb0 = xt_pool.tile([P, ET, SBLK], BF16, tag="xt")
                w_sb0 = w_pool.tile([P, ET, P], BF16, tag="w")
                for ch in range(8):
                    nc.sync.dma_start(w_sb0[:, ts(ch, ET // 8), :],
                                      wt_t[:, ts(ch, ET // 8), ts(0, P)])
                    nc.sync.dma_start(xt_sb0[:, ts(ch, ET // 8), :],
                                      xt_t[:, ts(ch, ET // 8), ts(0, SBLK)])
                xt_tiles[0] = xt_sb0
                w_tiles[(0, 0)] = w_sb0

                # warm up the CC stream while QKV computes (first collective
                # pays ~40us of cold-start cost)
                nc.gpsimd.collective_compute(
                    "AllGather", mybir.AluOpType.bypass,
                    replica_groups=[list(range(NCORES))],
                    ins=[warm_in.ap()], outs=[warm_out.ap()],
                )

                # constants (needed later than the first matmuls), issued on
                # the vector queue so the sync stream stays a pure xt/w feed
                rt_sb = cst.tile([D, D], BF16)
                nc.scalar.dma_start(rt_sb[:], rt_ext.ap())
                ones_sb = cst.tile([P, P], BF16)
                nc.scalar.dma_start(ones_sb[:], ones_ext.ap())
                ident_sb = cst.tile([P, P], BF16)
                nc.scalar.dma_start(ident_sb[:], ident_ext.ap())
                masks_sb = cst.tile([P, 4, SBLK], BF16)
                nc.scalar.dma_start(masks_sb[:], masks_t)

                for b in range(NSBLK):
                    sblk = ts(b, SBLK)
                    xt_sb = xt_tiles.pop(b)
                    cos_sb = cs_pool.tile([D, SBLK], F32, tag="cos")
                    nc.scalar.dma_start(cos_sb[:], cost_ext.ap()[:, sblk])
                    sin_sb = cs_pool.tile([D, SBLK], F32, tag="sin")
                    nc.scalar.dma_start(sin_sb[:], sins_ext.ap()[:, sblk])

                    for f in range(NF):
                        if f == 3 and b + 1 < NSBLK:
                            # prefetch the next seq block early so its first
                            # matmuls don't wait at the block boundary
                            nxt = xt_pool.tile([P, ET, SBLK], BF16, tag="xt")
                            for ch in range(4):
                                nc.sync.dma_start(
                                    nxt[:, ts(ch, ET // 4), :],
                                    xt_t[:, ts(ch, ET // 4), ts(b + 1, SBLK)])
                            xt_tiles[b + 1] = nxt
                        if (b, f) in w_tiles:
                            w_sb = w_tiles[(b, f)]
                        else:
                            w_sb = w_pool.tile([P, ET, P], BF16, tag="w")
                            nc.sync.dma_start(w_sb[:], wt_t[:, :, ts(f, P)])
                        acc_ps = ps_qkv.tile([P, SBLK], F32, tag="qkv")
                        for e in range(ET):
                            nc.tensor.matmul(
                                acc_ps[:], w_sb[:, e, :], xt_sb[:, e, :],
                                start=(e == 0), stop=(e == ET - 1),
                            )
                        if f < 2 * HPC:
                            # q/k: RoPE.  rotate-half = PE matmul with the
                            # swap matrix (sign lives in the signed-sin
                            # table).  A DMA-based partition swap is cheaper
                            # on paper but head-of-line-blocks the DMA and
                            # DVE queues, which costs far more than 512 PE
                            # columns.
                            qk_sb = qkc_pool.tile([P, SBLK], BF16, tag="qkc")
                            nc.any.tensor_copy(qk_sb[:], acc_ps[:])
                            rot_ps = ps_rot.tile([P, SBLK], F32, tag="rot")
                            nc.tensor.matmul(rot_ps[:], rt_sb[:], qk_sb[:],
                                             start=True, stop=True)
                            t1 = rtmp_pool.tile([P, SBLK], F32, tag="t1")
                            nc.vector.tensor_mul(out=t1[:], in0=acc_ps[:],
                                                 in1=cos_sb[:])
                            t2 = rtmp_pool.tile([P, SBLK], F32, tag="t2")
                            nc.vector.tensor_mul(out=t2[:], in0=rot_ps[:],
                                                 in1=sin_sb[:])
                            nc.vector.tensor_add(
                                out=ropeT_sb[:, f, sblk], in0=t1[:], in1=t2[:])
                        else:
                            # v: bf16 copy, then transpose the four 128x128
                            # tiles inline (keeps the PE warm vs. a separate
                            # transpose block between phases)
                            h = f - 2 * HPC
                            v_sb = qkc_pool.tile([P, SBLK], BF16, tag="vst")
                            nc.any.tensor_copy(v_sb[:], acc_ps[:])
                            for t in range(4):
                                vt_ps = ps_vtr.tile([P, P], BF16, tag="vtr")
                                nc.tensor.transpose(vt_ps[:],
                                                    v_sb[:, ts(t, P)],
                                                    ident_sb[:])
                                nc.any.tensor_copy(
                                    v_all_sb[:, h * KT + 4 * b + t, :],
                                    vt_ps[:])

            # --- Phase 2+3: attention per head, with the previous head's
            # o_proj pass interleaved at matmul granularity.  The o_proj
            # matmuls are pure-PE work that hides the attention phase's
            # Scalar (exp) and Vector (denominator) load; without the
            # interleave, attention is Scalar/DVE-bound and PE idles ~40%.
            with (
                tc.tile_pool(name="wot", bufs=1) as wot_pool,
                tc.tile_pool(name="attnT", bufs=1) as attnT_pool,
                tc.tile_pool(name="exp", bufs=8) as exp_pool,
                tc.tile_pool(name="dsum", bufs=3) as dsum_pool,
                tc.tile_pool(name="rcp", bufs=2) as rcp_pool,
                tc.tile_pool(name="at", bufs=4) as at_pool,
                tc.tile_pool(name="osb", bufs=3) as osb_pool,
                tc.tile_pool(name="part", bufs=1) as part_pool,
                tc.tile_pool(name="ps_sc", bufs=3, space="PSUM") as ps_sc,
                tc.tile_pool(name="ps_av", bufs=2, space="PSUM") as ps_av,
                tc.tile_pool(name="ps_den", bufs=1, space="PSUM") as ps_den,
                tc.tile_pool(name="ps_out", bufs=2, space="PSUM") as ps_out,
            ):
                wot_sb = wot_pool.tile([P, ET, ECOLS], BF16)
                nc.sync.dma_start(wot_sb[:], wot_t)
                attnT_sb = attnT_pool.tile([P, HPC, S], BF16)
                NST = S // P
                part_sb = part_pool.tile([P, NST, ECOLS], F32)

                def op_tile(p_h, st):
                    # one o_proj seq-tile of pass p_h: [128 seq, 512 e-cols],
                    # contracting the 8 128-row blocks of ccout[p_h][st//4]
                    a_sb = at_pool.tile([P, 8, P], BF16, tag="at")
                    nc.sync.dma_start(
                        a_sb[:], ccout_ts[p_h][st // 8][:, :, ts(st % 8, P)])
                    o_ps = ps_out.tile([P, ECOLS], F32, tag="out")
                    for c in range(8):
                        nc.tensor.matmul(o_ps[:], a_sb[:, c, :],
                                         wot_sb[:, 4 * c + p_h, :],
                                         start=(c == 0), stop=(c == 7))
                    if p_h == 0:
                        nc.vector.tensor_copy(part_sb[:, st, :], o_ps[:])
                    elif p_h < HPC - 1:
                        nc.vector.tensor_add(out=part_sb[:, st, :],
                                             in0=part_sb[:, st, :],
                                             in1=o_ps[:])
                    else:
                        o_sb = osb_pool.tile([P, ECOLS], F32, tag="osb")
                        nc.vector.tensor_add(out=o_sb[:], in0=o_ps[:],
                                             in1=part_sb[:, st, :])
                        # gpsimd queue: must not block a_sb loads on sync
                        nc.gpsimd.dma_start(out_ext.ap()[ts(st, P), :],
                                            o_sb[:])

                # attention head h has 40 score tiles; the 16 o_proj tiles
                # of pass h-1 slot in every other tile from tile 10 on (the
                # per-block AllGathers of head h-1 have all landed by then).
                OP_START = 10

                for h in range(HPC):
                    v_sb = v_all_sb[:, h * KT:(h + 1) * KT, :]
                    qh = ropeT_sb[:, h, :]
                    kh = ropeT_sb[:, HPC + h, :]
                    op_queue = [(h - 1, st) for st in range(NST)] if h else []
                    icount = 0
                    for j in range(NSBLK):
                        nkt = 4 * j + 4
                        av_ps = ps_av.tile([P, SBLK], F32, tag="av")
                        # bf16 accumulation is safe here: each dsum element
                        # sums at most 16 exp tiles (the heavy 2048-wide sum
                        # happens in the fp32-PSUM ones-matmul below), and
                        # bf16 runs the DVE in 2x mode.
                        dsum = dsum_pool.tile([P, SBLK], BF16, tag="dsum")
                        for i in range(nkt):
                            # diagonal tile r: columns below 128r are
                            # fully masked -> compute only [off:SBLK]
                            r = i - 4 * j
                            off = 128 * r if r > 0 else 0
                            qs = bass.ds(j * SBLK + off, SBLK - off)
                            sc_ps = ps_sc.tile([P, SBLK], F32, tag="sc")
                            nc.tensor.matmul(sc_ps[:, off:], kh[:, ts(i, P)],
                                             qh[:, qs],
                                             start=True, stop=True)
                            exp_sb = exp_pool.tile([P, SBLK], BF16,
                                                   tag="exp")
                            nc.scalar.activation(
                                exp_sb[:, off:], sc_ps[:, off:],
                                mybir.ActivationFunctionType.Exp,
                                scale=SCALE)
                            if r >= 0:
                                nc.vector.tensor_mul(
                                    out=exp_sb[:, off:],
                                    in0=exp_sb[:, off:],
                                    in1=masks_sb[:, r, off:])
                            # denominator partial sums accumulate on DVE
                            # (fp32); one matmul per (h, j) replicates the
                            # k-sum across partitions afterwards.
                            if i == 0:
                                nc.vector.tensor_copy(dsum[:], exp_sb[:])
                            else:
                                nc.vector.tensor_add(
                                    out=dsum[:, off:], in0=dsum[:, off:],
                                    in1=exp_sb[:, off:])
                            nc.tensor.matmul(
                                av_ps[:, off:], v_sb[:, i, :],
                                exp_sb[:, off:],
                                start=(i == 0), stop=(i == nkt - 1))
                            icount += 1
                            if (op_queue and icount >= OP_START
                                    and icount % 2 == 0):
                                op_tile(*op_queue.pop(0))
                        den_ps = ps_den.tile([P, SBLK], F32, tag="den")
                        nc.tensor.matmul(den_ps[:], ones_sb[:], dsum[:],
                                         start=True, stop=True)
                        recip_sb = rcp_pool.tile([P, SBLK], F32,
                                                 tag="rcp")
                        nc.vector.reciprocal_approx_fast(
                            out=recip_sb[:], in_=den_ps[:])
                        nc.vector.tensor_mul(
                            out=attnT_sb[:, h, ts(j, SBLK)],
                            in0=av_ps[:], in1=recip_sb[:])
                        if j % 2 == 1:
                            # ship this seq-half immediately: the AllGather
                            # overlaps the rest of the head, and o_proj can
                            # consume the final head's first half while the
                            # second is still in flight.
                            c = j // 2
                            nc.gpsimd.dma_start(
                                ccins[h][c].ap().rearrange(
                                    "(o p) s -> p o s", p=P),
                                attnT_sb[:, h:h + 1, ts(c, SCH)])
                            nc.gpsimd.collective_compute(
                                "AllGather", mybir.AluOpType.bypass,
                                replica_groups=[list(range(NCORES))],
                                ins=[ccins[h][c].ap()],
                                outs=[ccouts[h][c].ap()],
                            )

                    while op_queue:
                        op_tile(*op_queue.pop(0))

                # final o_proj pass (local head 3) after its AllGathers
                for st in range(NST):
                    op_tile(HPC - 1, st)

    nc.compile()
    _NC_CACHE = nc
    return nc


def _prep_inputs(hidden_states, cos, sin, w_pack, w_o):
    hs = np.asarray(hidden_states, dtype=np.float32).reshape(S, E)
    xt = np.ascontiguousarray(hs.T).astype(BF16NP)
    cost = np.ascontiguousarray(np.asarray(cos, dtype=np.float32).T)
    sint = np.ascontiguousarray(np.asarray(sin, dtype=np.float32).T)
    # signed sin table: rotate_half's sign folded in (rows 0..63 negated)
    sins = sint.copy()
    sins[:HALF] = -sins[:HALF]
    w_pack = np.asarray(w_pack, dtype=np.float32)
    w_o = np.asarray(w_o, dtype=np.float32)

    # rotate-half as a matmul: plain half-swap (sign lives in sins)
    R = np.zeros((D, D), dtype=np.float32)
    for dp in range(HALF):
        R[dp, dp + HALF] = 1.0
        R[dp + HALF, dp] = 1.0
    rt = np.ascontiguousarray(R.T).astype(BF16NP)

    masks = np.zeros((4, P, SBLK), dtype=np.float32)
    kk = np.arange(P)[:, None]
    qq = np.arange(SBLK)[None, :]
    for r in range(4):
        masks[r] = (P * r + kk <= qq).astype(np.float32)
    masks = masks.astype(BF16NP)

    ones = np.ones((P, P), dtype=BF16NP)
    ident = np.eye(P, dtype=np.float32).astype(BF16NP)

    in_maps = []
    hw = E // NCORES  # 512 head-rows per core in each of q/k/v
    for c in range(NCORES):
        rows = slice(c * hw, (c + 1) * hw)
        wqkv = np.concatenate(
            [w_pack[rows], w_pack[E:][rows], w_pack[2 * E:][rows]], axis=0)
        wt = np.ascontiguousarray(wqkv.T).astype(BF16NP)
        wot = np.ascontiguousarray(w_o[rows].T).astype(BF16NP)
        in_maps.append({
            "xt": xt, "wt": wt, "wot": wot,
            "cost": cost, "sins": sins, "rt": rt,
            "masks": masks, "ones": ones, "ident": ident,
        })
    return in_maps


def run(trace=False, trace_cores=None, **inputs):
    nc = build()
    in_maps = _prep_inputs(**inputs)
    res = run_bass_kernel_spmd(
        nc, in_maps, core_ids=list(range(NCORES)),
        trace=trace, trace_cores=trace_cores,
    )
    out = np.concatenate([res.results[c]["out"] for c in range(NCORES)], axis=1)
    return out.reshape(B, S, E).astype(np.float32), res


def kernel(**inputs) -> np.ndarray:
    out, _ = run(trace=False, **inputs)
    return out


# revision 37
# speedup vs baseline: 1.1477x; 1.0352x over previous
"""Baichuan attention layer (B=1, S=2048, E=4096, H=32, D=128) on 8 Trainium2
NeuronCores.

Sharding:
- QKV projection + RoPE + causal attention: tensor-parallel by head (4 heads
  per core). All per-head tensors live in transposed [feature, seq] layout so
  every matmul contracts over the partition dim with zero transposes:
    qkv^T[f, s]   = W @ X^T                (lhsT = W^T tiles, rhs = X^T tiles)
    scores^T[k,q] = K @ Q^T                (lhsT = K^T tile, rhs = Q^T block)
    att^T[d, q]   = V^T @ P^T              (lhsT = V tile,   rhs = exp tile)
  Softmax runs without max-subtraction (scores ~ N(0,1) after 1/sqrt(D), fp32
  exp is safe).  The denominator is accumulated on the Vector engine (exp
  tiles summed elementwise in fp32), then one all-ones [128,128] lhsT matmul
  per (head, q-block) replicates the k-sum across all PSUM partitions.
  RoPE's rotate-half is a partition swap done by an SBUF->SBUF DMA, with the
  sign folded into a host-precomputed signed-sin table; no PE matmul needed.
- One AllGather of att^T [512, 2048] bf16 per core -> full att^T [4096, 2048].
- o_proj: column-parallel (each core computes its 512 output columns for the
  full sequence, using its slice of w_o). Host concatenates along E.

All matmuls in bf16 with fp32 PSUM accumulation.
"""

import importlib.util
import sys
import types

import numpy as np
import ml_dtypes

BF16NP = ml_dtypes.bfloat16

B, S, E = 1, 2048, 4096
H, D = 32, 128
NCORES = 8
HPC = H // NCORES          # heads per core = 4
P = 128                    # partitions
SBLK = 512                 # seq block (matmul free dim)
NSBLK = S // SBLK          # 4
ET = E // P                # 32 e-tiles
NF = 3 * HPC               # 12 f-tiles per core (q0..3, k4..7, v8..11)
KT = S // P                # 16 k-tiles
ECOLS = E // NCORES        # 512 output columns per core
SCALE = 1.0 / float(np.sqrt(D))
HALF = D // 2


def _install_ntff_hook():
    """antenv.axon_hooks is absent in this image; recreate it from trn_boot's
    ctypes shim so run_bass_kernel_spmd(trace=True) can capture NTFF traces."""
    if "antenv.axon_hooks" in sys.modules:
        return
    try:
        spec = importlib.util.spec_from_file_location(
            "trn_boot", "/root/.axon_site/trn_agent_boot/trn_boot.py")
        tb = importlib.util.module_from_spec(spec)
        spec.loader.exec_module(tb)
        hook = tb._ntff_profile_via_ctypes("/opt/axon/libaxon_pjrt.so")
    except Exception:
        hook = None
    mod = types.ModuleType("antenv.axon_hooks")
    mod.get_axon_ntff_profile_hook = lambda: hook
    mod.set_axon_ntff_profile_hook = lambda h: None
    sys.modules["antenv.axon_hooks"] = mod


_install_ntff_hook()

import concourse.bass as bass  # noqa: E402
import concourse.mybir as mybir  # noqa: E402
import concourse.tile as tile  # noqa: E402
from concourse import bacc  # noqa: E402
from concourse.bass import ts  # noqa: E402
from concourse.bass_utils import run_bass_kernel_spmd  # noqa: E402


def _maybe_patch_ldw_opt():
    """walrus runs with --enable-ldw-opt=false by default; flipping it lets
    codegen pipeline LDWEIGHTS with matmuls.  Gated on BASS_LDW_OPT=1."""
    import os
    if os.environ.get("BASS_LDW_OPT") != "1":
        return
    from concourse import bass_utils as bu
    if getattr(bu, "_ldw_patched", False):
        return
    orig_run = bu.run_command

    def patched(cmd, *a, **kw):
        if isinstance(cmd, list):
            cmd = [c.replace("--enable-ldw-opt=false", "--enable-ldw-opt=true")
                   if isinstance(c, str) else c for c in cmd]
        return orig_run(cmd, *a, **kw)

    bu.run_command = patched
    bu._ldw_patched = True


_maybe_patch_ldw_opt()

BF16 = mybir.dt.bfloat16
F32 = mybir.dt.float32

_NC_CACHE = None


def build():
    global _NC_CACHE
    if _NC_CACHE is not None:
        return _NC_CACHE
    nc = bacc.Bacc("TRN2", target_bir_lowering=False, debug=False,
                   num_devices=NCORES)

    xt_ext = nc.dram_tensor("xt", [E, S], BF16, kind="ExternalInput")
    wt_ext = nc.dram_tensor("wt", [E, NF * P], BF16, kind="ExternalInput")
    wot_ext = nc.dram_tensor("wot", [E, ECOLS], BF16, kind="ExternalInput")
    cost_ext = nc.dram_tensor("cost", [D, S], F32, kind="ExternalInput")
    sins_ext = nc.dram_tensor("sins", [D, S], F32, kind="ExternalInput")
    rt_ext = nc.dram_tensor("rt", [D, D], BF16, kind="ExternalInput")
    masks_ext = nc.dram_tensor("masks", [4, P, SBLK], BF16, kind="ExternalInput")
    ones_ext = nc.dram_tensor("ones", [P, P], BF16, kind="ExternalInput")
    ident_ext = nc.dram_tensor("ident", [P, P], BF16, kind="ExternalInput")
    out_ext = nc.dram_tensor("out", [S, ECOLS], F32, kind="ExternalOutput")

    # Two AllGathers per local head (seq halves), issued as soon as each
    # half's attention output is ready: they overlap attention/o_proj
    # compute, and the split lets o_proj consume the final head's first
    # half while its second half is still in flight.  Each collective has
    # ~15us of fixed cost on the serial CC stream, so fewer+bigger wins;
    # a dummy warmup AllGather during QKV absorbs the ~40us cold start.
    # ccout[h][c] rank-r block = rows [128r, 128r+128) = global head 4r+h.
    NCH = 2
    SCH = S // NCH
    ccins = [[nc.dram_tensor(f"ccin{h}_{c}", [P, SCH], BF16)
              for c in range(NCH)] for h in range(HPC)]
    ccouts = [[nc.dram_tensor(f"ccout{h}_{c}", [NCORES * P, SCH], BF16,
                              addr_space="Shared") for c in range(NCH)]
              for h in range(HPC)]
    warm_in = nc.dram_tensor("warmin", [P, P], BF16)
    warm_out = nc.dram_tensor("warmout", [NCORES * P, P], BF16,
                              addr_space="Shared")

    xt_t = xt_ext.ap().rearrange("(eo p) s -> p eo s", p=P)
    wt_t = wt_ext.ap().rearrange("(eo p) f -> p eo f", p=P)
    wot_t = wot_ext.ap().rearrange("(fo p) e -> p fo e", p=P)
    masks_t = masks_ext.ap().rearrange("r p q -> p r q")
    # [p, c, s]: block c of ccout[h][j] = global head 4c + h
    ccout_ts = [[cc.ap().rearrange("(c p) s -> p c s", p=P) for cc in row]
                for row in ccouts]

    with tile.TileContext(nc) as tc:
        with (
            tc.tile_pool(name="cst", bufs=1) as cst,
            tc.tile_pool(name="ropeT", bufs=1) as ropeT_pool,
            tc.tile_pool(name="vall", bufs=1) as vall_pool,
        ):
            # q^T and k^T after RoPE: [128, 8, 2048]
            ropeT_sb = ropeT_pool.tile([P, 2 * HPC, S], BF16)
            # V tiles, transposed to [s, d] per 128x128 tile: [128, 64, 128]
            v_all_sb = vall_pool.tile([P, HPC * KT, P], BF16)

            # ---------------- Phase 1: QKV projection + RoPE -------------
            with (
                tc.tile_pool(name="xt", bufs=2) as xt_pool,
                tc.tile_pool(name="wq", bufs=5) as w_pool,
                tc.tile_pool(name="cs", bufs=2) as cs_pool,
                tc.tile_pool(name="qkc", bufs=3) as qkc_pool,
                tc.tile_pool(name="rtmp", bufs=2) as rtmp_pool,
                tc.tile_pool(name="ps_qkv", bufs=3, space="PSUM") as ps_qkv,
                tc.tile_pool(name="ps_rot", bufs=2, space="PSUM") as ps_rot,
                tc.tile_pool(name="ps_vtr", bufs=3, space="PSUM") as ps_vtr,
            ):
                # Pre-issue the b=0 input DMAs (chunked) so the first matmuls
                # start as early as possible; constants go afterwards.
                xt_tiles = {}
                w_tiles = {}
                xt_sb0 = xt_pool.tile([P, ET, SBLK], BF16, tag="xt")
                w_sb0 = w_pool.tile([P, ET, P], BF16, tag="w")
                for ch in range(8):
                    nc.sync.dma_start(w_sb0[:, ts(ch, ET // 8), :],
                                      wt_t[:, ts(ch, ET // 8), ts(0, P)])
                    nc.sync.dma_start(xt_sb0[:, ts(ch, ET // 8), :],
                                      xt_t[:, ts(ch, ET // 8), ts(0, SBLK)])
                xt_tiles[0] = xt_sb0
                w_tiles[(0, 0)] = w_sb0
                for f0 in (1, 2):
                    w_sbn = w_pool.tile([P, ET, P], BF16, tag="w")
                    for ch in range(4):
                        nc.sync.dma_start(w_sbn[:, ts(ch, ET // 4), :],
                                          wt_t[:, ts(ch, ET // 4), ts(f0, P)])
                    w_tiles[(0, f0)] = w_sbn

                # warm up the CC stream while QKV computes (first collective
                # pays ~40us of cold-start cost)
                nc.gpsimd.collective_compute(
                    "AllGather", mybir.AluOpType.bypass,
                    replica_groups=[list(range(NCORES))],
                    ins=[warm_in.ap()], outs=[warm_out.ap()],
                )

                # constants (needed later than the first matmuls), issued on
                # the vector queue so the sync stream stays a pure xt/w feed
                rt_sb = cst.tile([D, D], BF16)
                nc.scalar.dma_start(rt_sb[:], rt_ext.ap())
                ones_sb = cst.tile([P, P], BF16)
                nc.scalar.dma_start(ones_sb[:], ones_ext.ap())
                ident_sb = cst.tile([P, P], BF16)
                nc.scalar.dma_start(ident_sb[:], ident_ext.ap())
                masks_sb = cst.tile([P, 4, SBLK], BF16)
                nc.scalar.dma_start(masks_sb[:], masks_t)

                for b in range(NSBLK):
                    sblk = ts(b, SBLK)
                    xt_sb = xt_tiles.pop(b)
                    cos_sb = cs_pool.tile([D, SBLK], F32, tag="cos")
                    nc.scalar.dma_start(cos_sb[:], cost_ext.ap()[:, sblk])
                    sin_sb = cs_pool.tile([D, SBLK], F32, tag="sin")
                    nc.scalar.dma_start(sin_sb[:], sins_ext.ap()[:, sblk])

                    for f in range(NF):
                        if f == 3 and b + 1 < NSBLK:
                            # prefetch the next seq block early so its first
                            # matmuls don't wait at the block boundary
                            nxt = xt_pool.tile([P, ET, SBLK], BF16, tag="xt")
                            for ch in range(4):
                                nc.sync.dma_start(
                                    nxt[:, ts(ch, ET // 4), :],
                                    xt_t[:, ts(ch, ET // 4), ts(b + 1, SBLK)])
                            xt_tiles[b + 1] = nxt
                        if (b, f) in w_tiles:
                            w_sb = w_tiles[(b, f)]
                        else:
                            w_sb = w_pool.tile([P, ET, P], BF16, tag="w")
                            nc.sync.dma_start(w_sb[:], wt_t[:, :, ts(f, P)])
                        acc_ps = ps_qkv.tile([P, SBLK], F32, tag="qkv")
                        for e in range(ET):
                            nc.tensor.matmul(
                                acc_ps[:], w_sb[:, e, :], xt_sb[:, e, :],
                                start=(e == 0), stop=(e == ET - 1),
                            )
                        if f < 2 * HPC:
                            # q/k: RoPE.  rotate-half = PE matmul with the
                            # swap matrix (sign lives in the signed-sin
                            # table).  A DMA-based partition swap is cheaper
                            # on paper but head-of-line-blocks the DMA and
                            # DVE queues, which costs far more than 512 PE
                            # columns.
                            qk_sb = qkc_pool.tile([P, SBLK], BF16, tag="qkc")
                            nc.any.tensor_copy(qk_sb[:], acc_ps[:])
                            rot_ps = ps_rot.tile([P, SBLK], F32, tag="rot")
                            nc.tensor.matmul(rot_ps[:], rt_sb[:], qk_sb[:],
                                             start=True, stop=True)
                            t1 = rtmp_pool.tile([P, SBLK], F32, tag="t1")
                            nc.vector.tensor_mul(out=t1[:], in0=acc_ps[:],
                                                 in1=cos_sb[:])
                            t2 = rtmp_pool.tile([P, SBLK], F32, tag="t2")
                            nc.vector.tensor_mul(out=t2[:], in0=rot_ps[:],
                                                 in1=sin_sb[:])
                            nc.vector.tensor_add(
                                out=ropeT_sb[:, f, sblk], in0=t1[:], in1=t2[:])
                        else:
                            # v: bf16 copy, then transpose the four 128x128
                            # tiles inline (keeps the PE warm vs. a separate
                            # transpose block between phases)
                            h = f - 2 * HPC
                            v_sb = qkc_pool.tile([P, SBLK], BF16, tag="vst")
                            nc.any.tensor_copy(v_sb[:], acc_ps[:])
                            for t in range(4):
                                vt_ps = ps_vtr.tile([P, P], BF16, tag="vtr")
                                nc.tensor.transpose(vt_ps[:],
                                                    v_sb[:, ts(t, P)],
                                                    ident_sb[:])
                                nc.any.tensor_copy(
                                    v_all_sb[:, h * KT + 4 * b + t, :],
                                    vt_ps[:])

            # --- Phase 2+3: attention per head, with the previous head's
            # o_proj pass interleaved at matmul granularity.  The o_proj
            # matmuls are pure-PE work that hides the attention phase's
            # Scalar (exp) and Vector (denominator) load; without the
            # interleave, attention is Scalar/DVE-bound and PE idles ~40%.
            with (
                tc.tile_pool(name="wot", bufs=1) as wot_pool,
                tc.tile_pool(name="attnT", bufs=1) as attnT_pool,
                tc.tile_pool(name="exp", bufs=8) as exp_pool,
                tc.tile_pool(name="dsum", bufs=3) as dsum_pool,
                tc.tile_pool(name="rcp", bufs=2) as rcp_pool,
                tc.tile_pool(name="at", bufs=4) as at_pool,
                tc.tile_pool(name="osb", bufs=3) as osb_pool,
                tc.tile_pool(name="part", bufs=1) as part_pool,
                tc.tile_pool(name="ps_sc", bufs=3, space="PSUM") as ps_sc,
                tc.tile_pool(name="ps_av", bufs=2, space="PSUM") as ps_av,
                tc.tile_pool(name="ps_den", bufs=1, space="PSUM") as ps_den,
                tc.tile_pool(name="ps_out", bufs=2, space="PSUM") as ps_out,
            ):
                wot_sb = wot_pool.tile([P, ET, ECOLS], BF16)
                nc.sync.dma_start(wot_sb[:], wot_t)
                attnT_sb = attnT_pool.tile([P, HPC, S], BF16)
                NST = S // P
                part_sb = part_pool.tile([P, NST, ECOLS], F32)

                def op_tile(p_h, st):
                    # one o_proj seq-tile of pass p_h: [128 seq, 512 e-cols],
                    # contracting the 8 128-row blocks of ccout[p_h][st//4]
                    a_sb = at_pool.tile([P, 8, P], BF16, tag="at")
                    nc.sync.dma_start(
                        a_sb[:], ccout_ts[p_h][st // 8][:, :, ts(st % 8, P)])
                    o_ps = ps_out.tile([P, ECOLS], F32, tag="out")
                    for c in range(8):
                        nc.tensor.matmul(o_ps[:], a_sb[:, c, :],
                                         wot_sb[:, 4 * c + p_h, :],
                                         start=(c == 0), stop=(c == 7))
                    if p_h == 0:
                        nc.vector.tensor_copy(part_sb[:, st, :], o_ps[:])
                    elif p_h < HPC - 1:
                        nc.vector.tensor_add(out=part_sb[:, st, :],
                                             in0=part_sb[:, st, :],
                                             in1=o_ps[:])
                    else:
                        o_sb = osb_pool.tile([P, ECOLS], F32, tag="osb")
                        nc.vector.tensor_add(out=o_sb[:], in0=o_ps[:],
                                             in1=part_sb[:, st, :])
                        # gpsimd queue: must not block a_sb loads on sync
                        nc.gpsimd.dma_start(out_ext.ap()[ts(st, P), :],
                                            o_sb[:])

                # attention head h has 40 score tiles; the 16 o_proj tiles
                # of pass h-1 slot in every other tile from tile 12 on.
                # q-blocks run in order [2,3,0,1] so the second-half
                # AllGather (the one the next consumer needs first) ships
                # mid-head; o_proj consumes seq-tiles in the same order.
                OP_START = 12
                J_ORDER = [2, 3, 0, 1]
                ST_ORDER = list(range(8, 16)) + list(range(8))

                for h in range(HPC):
                    v_sb = v_all_sb[:, h * KT:(h + 1) * KT, :]
                    qh = ropeT_sb[:, h, :]
                    kh = ropeT_sb[:, HPC + h, :]
                    op_queue = ([(h - 1, st) for st in ST_ORDER]
                                if h else [])
                    icount = 0
                    for j in J_ORDER:
                        nkt = 4 * j + 4
                        av_ps = ps_av.tile([P, SBLK], F32, tag="av")
                        # bf16 accumulation is safe here: each dsum element
                        # sums at most 16 exp tiles (the heavy 2048-wide sum
                        # happens in the fp32-PSUM ones-matmul below), and
                        # bf16 runs the DVE in 2x mode.
                        dsum = dsum_pool.tile([P, SBLK], BF16, tag="dsum")
                        for i in range(nkt):
                            # diagonal tile r: columns below 128r are
                            # fully masked -> compute only [off:SBLK]
                            r = i - 4 * j
                            off = 128 * r if r > 0 else 0
                            qs = bass.ds(j * SBLK + off, SBLK - off)
                            sc_ps = ps_sc.tile([P, SBLK], F32, tag="sc")
                            nc.tensor.matmul(sc_ps[:, off:], kh[:, ts(i, P)],
                                             qh[:, qs],
                                             start=True, stop=True)
                            exp_sb = exp_pool.tile([P, SBLK], BF16,
                                                   tag="exp")
                            nc.scalar.activation(
                                exp_sb[:, off:], sc_ps[:, off:],
                                mybir.ActivationFunctionType.Exp,
                                scale=SCALE)
                            if r >= 0:
                                nc.vector.tensor_mul(
                                    out=exp_sb[:, off:],
                                    in0=exp_sb[:, off:],
                                    in1=masks_sb[:, r, off:])
                            # denominator partial sums accumulate on DVE
                            # (fp32); one matmul per (h, j) replicates the
                            # k-sum across partitions afterwards.
                            if i == 0:
                                nc.vector.tensor_copy(dsum[:], exp_sb[:])
                            else:
                                nc.vector.tensor_add(
                                    out=dsum[:, off:], in0=dsum[:, off:],
                                    in1=exp_sb[:, off:])
                            nc.tensor.matmul(
                                av_ps[:, off:], v_sb[:, i, :],
                                exp_sb[:, off:],
                                start=(i == 0), stop=(i == nkt - 1))
                            icount += 1
                            if (op_queue and icount >= OP_START
                                    and icount % 2 == 0):
                                op_tile(*op_queue.pop(0))
                        den_ps = ps_den.tile([P, SBLK], F32, tag="den")
                        nc.tensor.matmul(den_ps[:], ones_sb[:], dsum[:],
                                         start=True, stop=True)
                        recip_sb = rcp_pool.tile([P, SBLK], F32,
                                                 tag="rcp")
                        nc.vector.reciprocal_approx_fast(
                            out=recip_sb[:], in_=den_ps[:])
                        nc.vector.tensor_mul(
                            out=attnT_sb[:, h, ts(j, SBLK)],
                            in0=av_ps[:], in1=recip_sb[:])
                        if j % 2 == 1:
                            # ship this seq-half immediately: the AllGather
                            # overlaps the rest of the head, and o_proj can
                            # consume the final head's first half while the
                            # second is still in flight.
                            c = j // 2
                            nc.gpsimd.dma_start(
                                ccins[h][c].ap().rearrange(
                                    "(o p) s -> p o s", p=P),
                                attnT_sb[:, h:h + 1, ts(c, SCH)])
                            nc.gpsimd.collective_compute(
                                "AllGather", mybir.AluOpType.bypass,
                                replica_groups=[list(range(NCORES))],
                                ins=[ccins[h][c].ap()],
                                outs=[ccouts[h][c].ap()],
                            )

                    while op_queue:
                        op_tile(*op_queue.pop(0))

                # final o_proj pass (local head 3) after its AllGathers
                for st in ST_ORDER:
                    op_tile(HPC - 1, st)

    nc.compile()
    _NC_CACHE = nc
    return nc


def _prep_inputs(hidden_states, cos, sin, w_pack, w_o):
    hs = np.asarray(hidden_states, dtype=np.float32).reshape(S, E)
    xt = np.ascontiguousarray(hs.T).astype(BF16NP)
    cost = np.ascontiguousarray(np.asarray(cos, dtype=np.float32).T)
    sint = np.ascontiguousarray(np.asarray(sin, dtype=np.float32).T)
    # signed sin table: rotate_half's sign folded in (rows 0..63 negated)
    sins = sint.copy()
    sins[:HALF] = -sins[:HALF]
    w_pack = np.asarray(w_pack, dtype=np.float32)
    w_o = np.asarray(w_o, dtype=np.float32)

    # rotate-half as a matmul: plain half-swap (sign lives in sins)
    R = np.zeros((D, D), dtype=np.float32)
    for dp in range(HALF):
        R[dp, dp + HALF] = 1.0
        R[dp + HALF, dp] = 1.0
    rt = np.ascontiguousarray(R.T).astype(BF16NP)

    masks = np.zeros((4, P, SBLK), dtype=np.float32)
    kk = np.arange(P)[:, None]
    qq = np.arange(SBLK)[None, :]
    for r in range(4):
        masks[r] = (P * r + kk <= qq).astype(np.float32)
    masks = masks.astype(BF16NP)

    ones = np.ones((P, P), dtype=BF16NP)
    ident = np.eye(P, dtype=np.float32).astype(BF16NP)

    in_maps = []
    hw = E // NCORES  # 512 head-rows per core in each of q/k/v
    for c in range(NCORES):
        rows = slice(c * hw, (c + 1) * hw)
        wqkv = np.concatenate(
            [w_pack[rows], w_pack[E:][rows], w_pack[2 * E:][rows]], axis=0)
        wt = np.ascontiguousarray(wqkv.T).astype(BF16NP)
        wot = np.ascontiguousarray(w_o[rows].T).astype(BF16NP)
        in_maps.append({
            "xt": xt, "wt": wt, "wot": wot,
            "cost": cost, "sins": sins, "rt": rt,
            "masks": masks, "ones": ones, "ident": ident,
        })
    return in_maps


def run(trace=False, trace_cores=None, **inputs):
    nc = build()
    in_maps = _prep_inputs(**inputs)
    res = run_bass_kernel_spmd(
        nc, in_maps, core_ids=list(range(NCORES)),
        trace=trace, trace_cores=trace_cores,
    )
    out = np.concatenate([res.results[c]["out"] for c in range(NCORES)], axis=1)
    return out.reshape(B, S, E).astype(np.float32), res


def kernel(**inputs) -> np.ndarray:
    out, _ = run(trace=False, **inputs)
    return out
